# revision 1
# baseline (speedup 1.0000x reference)
"""GPT-style dense transformer on 8 Trainium2 NeuronCores.

Sharding: token-parallel. Core c owns positions t = 8*i + c of BOTH batches
(256 positions per batch at full size -> 512 tokens per core). All per-token
work (LN, qkv, out_proj, ff, lm_head) is local; attention needs all keys, so
K^T and V are AllGathered across the 8 cores once per layer (bf16, ~1.5MB per
rank). The strided assignment makes every core's causal structure identical
(block-lower-triangular over local indices, with a per-source-core diagonal
rule c' <= c shipped as a data mask), so one SPMD program serves all cores.

Layout trick: the residual stream lives TRANSPOSED in SBUF as x^T [D, tokens].
Every matmul then consumes natural-layout weights as the stationary operand
and transposed activations as the moving operand, producing transposed
activations again -- zero on-device transposes after the embedding load.
Attention is computed as S^T = K^T.T @ Q^T (scores with keys on partitions),
exp'd without max-subtraction (scores are bounded ~|0.3| by construction),
masked multiplicatively, and y^T = (V|1).T @ expS^T accumulates both the
numerator and the softmax denominator (ones column) in one PSUM pass.
LN scale/bias are folded into the adjacent weights on the host.
"""

import sys

for _p in ("/opt/trn_rl_repo",):
    if _p not in sys.path:
        sys.path.insert(0, _p)

import numpy as np
import ml_dtypes

import concourse.bass as bass
import concourse.bacc as bacc
import concourse.mybir as mybir
import concourse.tile as tile
from concourse.masks import make_identity

BF16 = mybir.dt.bfloat16
F32 = mybir.dt.float32
I32 = mybir.dt.int32
AF = mybir.ActivationFunctionType
ALU = mybir.AluOpType

NCORES = 8
H = 12          # heads
HD = 64         # head dim
D = 768
D3 = 3 * D      # 2304
DF = 4 * D      # 3072
KD = D // 128   # 6 d-tiles
EPS = 1e-5

bf16 = ml_dtypes.bfloat16


def build_nc(nb, L, V, stop_at=None):
    """Build the SPMD Bass module. nb = 128-token tiles per (core, batch).
    Full size: nb=2 -> 512 tokens/core, T = 8*128*nb = 2048."""
    NT = 2 * nb          # token tiles per core
    PT = NT * 128        # tokens per core
    NVC = (V + 511) // 512  # vocab chunks for lm_head

    nc = bacc.Bacc("TRN2", target_bir_lowering=False, num_devices=NCORES)

    # ---- I/O ----
    idxs = nc.dram_tensor("idxs", [128, NT], I32, kind="ExternalInput")
    posT = nc.dram_tensor("posT", [D, PT], F32, kind="ExternalInput")
    masks = nc.dram_tensor("masks", [128, NCORES * 128], BF16, kind="ExternalInput")
    toke = nc.dram_tensor("toke", [V, D], F32, kind="ExternalInput")
    embT = nc.dram_tensor("embT", [D, V], BF16, kind="ExternalInput")
    wqkv = [nc.dram_tensor(f"wqkv{l}", [D, D3], BF16, kind="ExternalInput") for l in range(L)]
    bqkv = [nc.dram_tensor(f"bqkv{l}", [128, 12], F32, kind="ExternalInput") for l in range(L)]
    bqv = [nc.dram_tensor(f"bqv{l}", [1, D], F32, kind="ExternalInput") for l in range(L)]
    wout = [nc.dram_tensor(f"wout{l}", [D, D], BF16, kind="ExternalInput") for l in range(L)]
    w1 = [nc.dram_tensor(f"w1_{l}", [D, DF], BF16, kind="ExternalInput") for l in range(L)]
    b1 = [nc.dram_tensor(f"b1_{l}", [128, 24], F32, kind="ExternalInput") for l in range(L)]
    w2 = [nc.dram_tensor(f"w2_{l}", [DF, D], BF16, kind="ExternalInput") for l in range(L)]
    logits = nc.dram_tensor("logits", [PT, V], F32, kind="ExternalOutput")

    from contextlib import ExitStack
    with tile.TileContext(nc) as tc, ExitStack() as ctx:
        def pool(**kw):
            return ctx.enter_context(tc.tile_pool(**kw))
        # ---- pools ----
        const = pool(name="const", bufs=1)
        resid = pool(name="resid", bufs=1)
        acts = pool(name="acts", bufs=1)
        kvres = pool(name="kvres", bufs=1)
        wpool = pool(name="wpool", bufs=1)
        biasp = pool(name="biasp", bufs=2)
        rot = pool(name="rot", bufs=2)
        esp = pool(name="esp", bufs=6)
        gp = pool(name="gp", bufs=4)
        embp = pool(name="embp", bufs=2)
        logp = pool(name="logp", bufs=3)
        rowp = pool(name="rowp", bufs=4)
        ps_s = pool(name="ps_s", bufs=2, space="PSUM")
        ps_y = pool(name="ps_y", bufs=4, space="PSUM")
        ps_m = pool(name="ps_m", bufs=2, space="PSUM")
        dram = pool(name="dram", bufs=2, space="DRAM")

        # ---- constants ----
        ident = const.tile([128, 128], F32, name="ident", tag="ident")
        make_identity(nc, ident)
        ones_col = const.tile([128, 1], BF16, name="ones_col", tag="ones_col")
        nc.gpsimd.memset(ones_col[:, :], 1.0)
        ones_row = const.tile([1, 128], F32, name="ones_row", tag="ones_row")
        nc.gpsimd.memset(ones_row[:, :], 1.0)
        eps_t = const.tile([1, 1], F32, name="eps_t", tag="eps_t")
        nc.gpsimd.memset(eps_t[:, :], EPS)
        zero_col = const.tile([128, 1], F32, name="zero_col", tag="zero_col")
        nc.gpsimd.memset(zero_col[:, :], 0.0)
        mask_sb = const.tile([128, NCORES * 128], BF16, name="mask_sb", tag="mask_sb")
        nc.sync.dma_start(out=mask_sb[:, :], in_=masks[:, :])
        idx_sb = const.tile([128, NT], I32, name="idx_sb", tag="idx_sb")
        nc.sync.dma_start(out=idx_sb[:, :], in_=idxs[:, :])

        # ---- persistent per-layer state ----
        xT = [resid.tile([128, PT], F32, name=f"xt{d}", tag=f"xt{d}") for d in range(KD)]
        hT = [acts.tile([128, PT], BF16, name=f"ht{d}", tag=f"ht{d}") for d in range(KD)]
        qT = [acts.tile([128, PT], BF16, name=f"qt{d}", tag=f"qt{d}") for d in range(KD)]
        yT = [acts.tile([128, PT], BF16, name=f"yt{d}", tag=f"yt{d}") for d in range(KD)]
        # gathered K^T (half the heads at a time): [c'][r] covering 384 rows
        ktg = [[kvres.tile([128, PT], BF16, name=f"kt{c}_{r}", tag=f"kt{c}_{r}")
                for r in range(3)] for c in range(NCORES)]
        # gathered V, padded per head with a ones column: [c'][ktile]
        vg = [[kvres.tile([128, 6 * 65], BF16, name=f"v{c}_{t}", tag=f"v{c}_{t}")
               for t in range(NT)] for c in range(NCORES)]
        for c in range(NCORES):
            for t in range(NT):
                nc.gpsimd.memset(vg[c][t][:, :].rearrange("p (s e) -> p s e", e=65)[:, :, 64:65], 1.0)

        wbig = [wpool.tile([128, DF], BF16, name=f"wb{d}", tag=f"wb{d}") for d in range(KD)]
        w768 = [wpool.tile([128, D], BF16, name=f"w7{i}", tag=f"w7{i}") for i in range(KD)]

        def layernorm_T(dst_bf16):
            """dst[d] <- normalize(xT) across the D (partition-tiled) axis."""
            s1 = ps_m.tile([1, PT], F32, name="s1", tag="m")
            s2 = ps_m.tile([1, PT], F32, name="s2", tag="m")
            for d in range(KD):
                xb = rot.tile([128, PT], BF16, name="xb", tag="xb")
                nc.vector.tensor_copy(out=xb[:, :], in_=xT[d][:, :])
                sq = rot.tile([128, PT], BF16, name="sq", tag="sq")
                nc.vector.tensor_mul(out=sq[:, :], in0=xb[:, :], in1=xb[:, :])
                nc.tensor.matmul(out=s1[:, :], lhsT=ones_col[:, :], rhs=xb[:, :],
                                 start=(d == 0), stop=(d == KD - 1))
                nc.tensor.matmul(out=s2[:, :], lhsT=ones_col[:, :], rhs=sq[:, :],
                                 start=(d == 0), stop=(d == KD - 1))
            mrow = rowp.tile([1, PT], F32, name="mrow", tag="row")
            nc.vector.tensor_scalar(out=mrow[:, :], in0=s1[:, :], scalar1=1.0 / D,
                                    scalar2=None, op0=ALU.mult)
            vrow = rowp.tile([1, PT], F32, name="vrow", tag="row")
            nc.vector.tensor_scalar(out=vrow[:, :], in0=s2[:, :], scalar1=1.0 / D,
                                    scalar2=None, op0=ALU.mult)
            msq = rowp.tile([1, PT], F32, name="msq", tag="row")
            nc.vector.tensor_mul(out=msq[:, :], in0=mrow[:, :], in1=mrow[:, :])
            nc.vector.tensor_sub(out=vrow[:, :], in0=vrow[:, :], in1=msq[:, :])
            srow = rowp.tile([1, PT], F32, name="srow", tag="row")
            nc.scalar.activation(out=srow[:, :], in_=vrow[:, :], func=AF.Sqrt,
                                 bias=eps_t[:, :])
            rrow = rowp.tile([1, PT], F32, name="rrow", tag="row")
            nc.vector.reciprocal(out=rrow[:, :], in_=srow[:, :])
            mr = rowp.tile([1, PT], F32, name="mr", tag="row")
            nc.vector.tensor_mul(out=mr[:, :], in0=mrow[:, :], in1=rrow[:, :])
            # broadcast [1, PT] rows across 128 partitions via K=1 matmul
            bc_r = ps_m.tile([128, PT], F32, name="bc_r", tag="m")
            nc.tensor.matmul(out=bc_r[:, :], lhsT=ones_row[:, :], rhs=rrow[:, :],
                             start=True, stop=True)
            bc_mr = ps_m.tile([128, PT], F32, name="bc_mr", tag="m")
            nc.tensor.matmul(out=bc_mr[:, :], lhsT=ones_row[:, :], rhs=mr[:, :],
                             start=True, stop=True)
            for d in range(KD):
                t32 = rot.tile([128, PT], F32, name="t32", tag="t32")
                nc.vector.tensor_mul(out=t32[:, :], in0=xT[d][:, :], in1=bc_r[:, :])
                nc.vector.tensor_sub(out=dst_bf16[d][:, :], in0=t32[:, :], in1=bc_mr[:, :])

        # ================= embedding =================
        # pos arrives pre-transposed; add it after the on-chip transpose so the
        # join is PE+one-DMA only (sync-wait slot limits).
        # aliased into the weight-slab slots (same tags) -- embed finishes
        # before the first qkv weight DMA needs them
        posv = [wpool.tile([128, PT], F32, name=f"posv{d}", tag=f"wb{d}")
                for d in range(KD)]
        for d in range(KD):
            nc.sync.dma_start(out=posv[d][:, :], in_=posT[d * 128:(d + 1) * 128, :])
        for tt in range(NT):
            xg = rot.tile([128, D], F32, name="xg", tag="xg", bufs=2)
            nc.gpsimd.indirect_dma_start(
                out=xg[:, :], out_offset=None, in_=toke[:, :],
                in_offset=bass.IndirectOffsetOnAxis(ap=idx_sb[:, tt:tt + 1], axis=0))
            for d in range(KD):
                tp = ps_s.tile([128, 128], F32, name="tp", tag="s")
                nc.tensor.transpose(out=tp[:, :], in_=xg[:, d * 128:(d + 1) * 128],
                                    identity=ident[:, :])
                nc.vector.tensor_tensor(
                    out=xT[d][:, tt * 128:(tt + 1) * 128], in0=tp[:, :],
                    in1=posv[d][:, tt * 128:(tt + 1) * 128], op=ALU.add)

        # ================= layers =================
        for l in range(L):
            last = l == L - 1
            def _stop(tag):
                return last and stop_at == tag
            # ---- LN1 -> hT ----
            layernorm_T(hT)
            if _stop("ln1"):
                return nc

            bq = biasp.tile([128, 12], F32, name="bq", tag="bq")
            nc.sync.dma_start(out=bq[:, :], in_=bqkv[l][:, :])
            bv = biasp.tile([1, D], F32, name="bv", tag="bv")
            nc.sync.dma_start(out=bv[:, :], in_=bqv[l][:, :])

            # ---- qkv: Q^T, K^T (transposed out), V (natural out) ----
            for d in range(KD):
                nc.sync.dma_start(out=wbig[d][:, :D3], in_=wqkv[l][d * 128:(d + 1) * 128, :])
            kv_in = dram.tile([2 * D, PT], BF16, name="kv_in", tag="kv_in")
            for ot in range(12):  # 0..5 Q^T, 6..11 K^T
                ps = ps_s.tile([128, PT], F32, name="ps_qk", tag="s")
                for d in range(KD):
                    nc.tensor.matmul(out=ps[:, :], lhsT=wbig[d][:, ot * 128:(ot + 1) * 128],
                                     rhs=hT[d][:, :], start=(d == 0), stop=(d == KD - 1))
                if ot < KD:
                    nc.vector.tensor_scalar(out=qT[ot][:, :], in0=ps[:, :],
                                            scalar1=bq[:, ot:ot + 1], scalar2=None, op0=ALU.add)
                else:
                    klo = rot.tile([128, PT], BF16, name="klo", tag="klo", bufs=3)
                    nc.vector.tensor_scalar(out=klo[:, :], in0=ps[:, :],
                                            scalar1=bq[:, ot:ot + 1], scalar2=None, op0=ALU.add)
                    r = ot - KD
                    nc.sync.dma_start(out=kv_in[r * 128:(r + 1) * 128, :], in_=klo[:, :])
            kv_flat = kv_in[:, :].rearrange("r c -> (r c)")
            # broadcast the V bias [1, D] across partitions once per layer
            bvb = rot.tile([128, D], F32, name="bvb", tag="bvb")
            for vh in range(2):
                bcv = ps_m.tile([128, 384], F32, name="bcv", tag="m")
                nc.tensor.matmul(out=bcv[:, :], lhsT=ones_row[:, :],
                                 rhs=bv[:, vh * 384:(vh + 1) * 384], start=True, stop=True)
                nc.vector.tensor_copy(out=bvb[:, vh * 384:(vh + 1) * 384], in_=bcv[:, :])
            for tt in range(NT):
                vloc = rot.tile([128, D], BF16, name="vloc", tag="vloc", bufs=3)
                for vh in range(2):
                    ps = ps_y.tile([128, 384], F32, name="ps_v", tag="y")
                    for d in range(KD):
                        nc.tensor.matmul(
                            out=ps[:, :],
                            lhsT=hT[d][:, tt * 128:(tt + 1) * 128],
                            rhs=wbig[d][:, D3 - D + vh * 384: D3 - D + (vh + 1) * 384],
                            start=(d == 0), stop=(d == KD - 1))
                    nc.vector.tensor_add(out=vloc[:, vh * 384:(vh + 1) * 384],
                                         in0=ps[:, :], in1=bvb[:, vh * 384:(vh + 1) * 384])
                nc.sync.dma_start(
                    out=kv_flat[D * PT + tt * 128 * D: D * PT + (tt + 1) * 128 * D]
                    .rearrange("(p e) -> p e", p=128),
                    in_=vloc[:, :])
            if _stop("qkv"):
                return nc

            # ---- AllGather K^T,V across all 8 cores ----
            kv_out = dram.tile([NCORES * 2 * D, PT], BF16, name="kv_out", tag="kv_out", addr_space="Shared")
            nc.gpsimd.collective_compute(
                "AllGather", ALU.bypass,
                replica_groups=[list(range(NCORES))],
                ins=[kv_in[:, :].opt()], outs=[kv_out[:, :].opt()])
            kvo_flat = kv_out[:, :].rearrange("r c -> (r c)")
            if _stop("ag"):
                return nc

            # ---- attention, half the heads at a time ----
            for half in range(2):
                for c in range(NCORES):
                    for r in range(3):
                        nc.sync.dma_start(
                            out=ktg[c][r][:, :],
                            in_=kv_out[c * 2 * D + half * 384 + r * 128:
                                       c * 2 * D + half * 384 + (r + 1) * 128, :])
                    for t in range(NT):
                        src = (kvo_flat[(c * 2 * D + D) * PT + t * 128 * D:
                                        (c * 2 * D + D) * PT + (t + 1) * 128 * D]
                               .rearrange("(p s e) -> p s e", p=128, e=64))
                        nc.sync.dma_start(
                            out=vg[c][t][:, :].rearrange("p (s e) -> p s e", e=65)[:, :, 0:64],
                            in_=src[:, half * 6:(half + 1) * 6, :])
                for h in range(half * 6, half * 6 + 6):
                    hs = h - half * 6
                    kr = (h * 64 - half * 384) // 128
                    kp = (h * 64) % 128
                    qtile = qT[h // 2]
                    qp = (h % 2) * 64
                    y_ps = [ps_y.tile([65, nb * 128], F32, name=f"y_ps{b}", tag="y")
                            for b in range(2)]
                    for c in range(NCORES):
                        for b in range(2):
                            for j in range(nb):
                                N = (nb - j) * 128
                                col0 = (b * nb + j) * 128
                                s_ps = ps_s.tile([128, N], F32, name="s_ps", tag="s")
                                nc.tensor.matmul(
                                    out=s_ps[:, :],
                                    lhsT=ktg[c][kr][kp:kp + 64, col0:col0 + 128],
                                    rhs=qtile[qp:qp + 64, col0:(b + 1) * nb * 128],
                                    start=True, stop=True)
                                es = esp.tile([128, N], BF16, name="es", tag="es")
                                nc.scalar.activation(out=es[:, :], in_=s_ps[:, :],
                                                     func=AF.Exp, bias=zero_col[:, :],
                                                     scale=0.125)
                                # masked diagonal block goes out-of-place so each
                                # att@V matmul depends on a single compute engine
                                esm = esp.tile([128, 128], BF16, name="esm", tag="esm")
                                nc.vector.tensor_mul(
                                    out=esm[:, :], in0=es[:, 0:128],
                                    in1=mask_sb[:, c * 128:(c + 1) * 128])
                                vh_ap = vg[c][b * nb + j][:, hs * 65:(hs + 1) * 65]
                                first = c == 0 and j == 0
                                last = c == NCORES - 1 and j == nb - 1
                                nc.tensor.matmul(
                                    out=y_ps[b][:, j * 128:(j + 1) * 128],
                                    lhsT=vh_ap, rhs=esm[:, :],
                                    start=first, stop=last and N == 128)
                                if N > 128:
                                    nc.tensor.matmul(
                                        out=y_ps[b][:, (j + 1) * 128:],
                                        lhsT=vh_ap, rhs=es[:, 128:],
                                        start=False, stop=last)
                    for b in range(2):
                        zrec = rowp.tile([1, nb * 128], F32, name="zrec", tag="row")
                        nc.vector.reciprocal(out=zrec[:, :], in_=y_ps[b][64:65, :])
                        bc = ps_m.tile([64, nb * 128], F32, name="bc", tag="m")
                        nc.tensor.matmul(out=bc[:, :], lhsT=ones_row[:, 0:64],
                                         rhs=zrec[:, :], start=True, stop=True)
                        bcs = rot.tile([64, nb * 128], F32, name="bcs", tag="bcs")
                        nc.vector.tensor_copy(out=bcs[:, :], in_=bc[:, :])
                        nc.vector.tensor_tensor(
                            out=yT[h // 2][qp:qp + 64, b * nb * 128:(b + 1) * nb * 128],
                            in0=y_ps[b][0:64, :], in1=bcs[:, :], op=ALU.mult)

            if _stop("attn"):
                return nc
            # ---- out_proj + residual: xT += Wout^T y^T ----
            for k in range(KD):
                nc.sync.dma_start(out=w768[k][:, :], in_=wout[l][k * 128:(k + 1) * 128, :])
            oacc = [ (ps_y if o < 4 else ps_m).tile([128, PT], F32, name=f"oacc{o}",
                                                    tag=("y" if o < 4 else "m"))
                     for o in range(KD)]
            for k in range(KD):
                for o in range(KD):
                    nc.tensor.matmul(out=oacc[o][:, :], lhsT=w768[k][:, o * 128:(o + 1) * 128],
                                     rhs=yT[k][:, :], start=(k == 0), stop=(k == KD - 1))
            for o in range(KD):
                nc.vector.tensor_add(out=xT[o][:, :], in0=xT[o][:, :], in1=oacc[o][:, :])
            if _stop("proj"):
                return nc

            # ---- LN2 -> hT ----
            layernorm_T(hT)

            # ---- FF: g^T tile-by-tile, immediately consumed into ff2 accumulators ----
            bft = biasp.tile([128, 24], F32, name="bft", tag="bft")
            nc.sync.dma_start(out=bft[:, :], in_=b1[l][:, :])
            for d in range(KD):
                nc.sync.dma_start(out=wbig[d][:, :], in_=w1[l][d * 128:(d + 1) * 128, :])
            facc = [ (ps_y if o < 4 else ps_m).tile([128, PT], F32, name=f"facc{o}",
                                                    tag=("y" if o < 4 else "m"))
                     for o in range(KD)]
            for ot in range(24):
                ps = ps_s.tile([128, PT], F32, name="ps_f1", tag="s")
                for d in range(KD):
                    nc.tensor.matmul(out=ps[:, :], lhsT=wbig[d][:, ot * 128:(ot + 1) * 128],
                                     rhs=hT[d][:, :], start=(d == 0), stop=(d == KD - 1))
                g = gp.tile([128, PT], BF16, name="g", tag="g")
                nc.scalar.activation(out=g[:, :], in_=ps[:, :], func=AF.Gelu,
                                     bias=bft[:, ot:ot + 1], scale=1.0)
                wslab = wpool.tile([128, D], BF16, name="w2s", tag="w2s", bufs=4)
                nc.sync.dma_start(out=wslab[:, :], in_=w2[l][ot * 128:(ot + 1) * 128, :])
                for o in range(KD):
                    nc.tensor.matmul(out=facc[o][:, :], lhsT=wslab[:, o * 128:(o + 1) * 128],
                                     rhs=g[:, :], start=(ot == 0), stop=(ot == 23))
            for o in range(KD):
                nc.vector.tensor_add(out=xT[o][:, :], in0=xT[o][:, :], in1=facc[o][:, :])

        # ================= final LN + lm_head =================
        layernorm_T(hT)
        for vc in range(NVC):
            nv = min(512, V - vc * 512)
            esl = [embp.tile([128, 512], BF16, name=f"esl{d}", tag=f"em{d}") for d in range(KD)]
            for d in range(KD):
                nc.sync.dma_start(out=esl[d][:, 0:nv],
                                  in_=embT[d * 128:(d + 1) * 128, vc * 512:vc * 512 + nv])
            for tt in range(NT):
                ps = ps_s.tile([128, 512], F32, name="ps_lm", tag="s")
                for d in range(KD):
                    nc.tensor.matmul(out=ps[:, 0:nv],
                                     lhsT=hT[d][:, tt * 128:(tt + 1) * 128],
                                     rhs=esl[d][:, 0:nv], start=(d == 0), stop=(d == KD - 1))
                lsb = logp.tile([128, 512], F32, name="lsb", tag="lsb")
                nc.vector.tensor_copy(out=lsb[:, 0:nv], in_=ps[:, 0:nv])
                nc.sync.dma_start(out=logits[tt * 128:(tt + 1) * 128, vc * 512:vc * 512 + nv],
                                  in_=lsb[:, 0:nv])
    nc.finalize()
    return nc


# ------------------------------------------------------------------
# host side
# ------------------------------------------------------------------

def _prep_inputs(nb, L, V, idx, tok_emb, pos_emb, ln1_w, ln1_b, qkv_w, out_w,
                 ln2_w, ln2_b, ff1_w, ff2_w, lnf_w, lnf_b):
    NT = 2 * nb
    PT = NT * 128
    T = 8 * nb * 128
    idx = np.asarray(idx).astype(np.int32)
    f = np.asarray

    shared = {
        "toke": f(tok_emb, dtype=np.float32),
        "embT": np.ascontiguousarray((f(tok_emb, dtype=np.float32) * f(lnf_w, dtype=np.float32)[None, :]).T).astype(bf16),
    }
    for l in range(L):
        wq = f(qkv_w[l], dtype=np.float32) * f(ln1_w[l], dtype=np.float32)[:, None]
        bq_full = f(ln1_b[l], dtype=np.float32) @ f(qkv_w[l], dtype=np.float32)  # [3D]
        shared[f"wqkv{l}"] = wq.astype(bf16)
        shared[f"bqkv{l}"] = np.ascontiguousarray(bq_full[:12 * 128].reshape(12, 128).T).astype(np.float32)
        shared[f"bqv{l}"] = bq_full[2 * D:].reshape(1, D).astype(np.float32)
        shared[f"wout{l}"] = f(out_w[l], dtype=np.float32).astype(bf16)
        w1e = f(ff1_w[l], dtype=np.float32) * f(ln2_w[l], dtype=np.float32)[:, None]
        b1_full = f(ln2_b[l], dtype=np.float32) @ f(ff1_w[l], dtype=np.float32)  # [4D]
        shared[f"w1_{l}"] = w1e.astype(bf16)
        shared[f"b1_{l}"] = np.ascontiguousarray(b1_full.reshape(24, 128).T).astype(np.float32)
        shared[f"w2_{l}"] = f(ff2_w[l], dtype=np.float32).astype(bf16)

    pos_f = f(pos_emb, dtype=np.float32)
    in_maps = []
    for c in range(NCORES):
        m = dict(shared)
        L_loc = np.arange(PT)
        b_loc = L_loc // (nb * 128)
        t_loc = 8 * (L_loc % (nb * 128)) + c
        idx_core = idx[b_loc, t_loc]  # [PT]
        m["idxs"] = np.ascontiguousarray(idx_core.reshape(NT, 128).T).astype(np.int32)
        m["posT"] = np.ascontiguousarray(pos_f[t_loc].T).astype(np.float32)
        mk = np.zeros((128, NCORES * 128), dtype=np.float32)
        for cp in range(NCORES):
            mk[:, cp * 128:(cp + 1) * 128] = np.triu(np.ones((128, 128), np.float32),
                                                     0 if cp <= c else 1)
        m["masks"] = mk.astype(bf16)
        in_maps.append(m)
    return in_maps


_NC_CACHE = {}


def _get_nc(nb, L, V):
    key = (nb, L, V)
    if key not in _NC_CACHE:
        _NC_CACHE[key] = build_nc(nb, L, V)
    return _NC_CACHE[key]


def run_on_hw(nb, L, V, inputs, trace=False):
    from concourse import bass_utils
    nc = _get_nc(nb, L, V)
    in_maps = _prep_inputs(nb, L, V, **inputs)
    res = bass_utils.run_bass_kernel_spmd(nc, in_maps, core_ids=list(range(NCORES)),
                                          trace=trace)
    return res


def assemble(nb, L, V, results, lnf_b, tok_emb):
    NT = 2 * nb
    PT = NT * 128
    T = 8 * nb * 128
    out = np.empty((2, T, V), dtype=np.float32)
    for c in range(NCORES):
        lg = results[c]["logits"].reshape(2, nb * 128, V)
        out[:, c::8, :] = lg
    lnf_b = np.asarray(lnf_b, dtype=np.float32)
    if np.any(lnf_b):
        out += (lnf_b @ np.asarray(tok_emb, dtype=np.float32).T)[None, None, :]
    return out


def kernel(**inputs):
    nb, L, V = 2, 6, 32000
    res = run_on_hw(nb, L, V, inputs)
    return assemble(nb, L, V, res.results, inputs["lnf_b"], inputs["tok_emb"])



# revision 14
# speedup vs baseline: 1.3262x; 1.3262x over previous
"""GPT-style dense transformer on 8 Trainium2 NeuronCores (v2).

Sharding: token-parallel. Core c owns positions t = 8*i + c of BOTH batches
(256 positions per batch -> 512 tokens per core). All per-token work (LN,
qkv, out_proj, ff, lm_head) is local; attention needs all keys, so K^T and V
are AllGathered across the 8 cores once per layer. The strided assignment
makes every core's causal structure identical (block-lower-triangular over
local indices, with a per-source-core diagonal mask), so one SPMD program
serves all cores.

v2 perf structure (vs v1):
- Attention scores are computed per HEAD PAIR: heads (2m, 2m+1) occupy PE
  rows 0:64 / 64:128 (K=64 each). Adjacent issue with disjoint row groups
  lets the PE run both concurrently (~2x on score matmuls).
- Score PSUM tiles span 2 banks [128, 1024]: bank0 = batch0, bank1 = batch1;
  per (head, src-core) ONE strided exp [128, 2, 384] on the ACT engine.
- Causal diagonal masks are applied multiplicatively on the GPSIMD engine
  (otherwise idle), one strided [128, 2, 2, 128] multiply per (head, core).
- The K/V AllGather is split: AG(K) is issued right after the K projection
  (overlaps Q/V compute), AG(V) after V (overlaps early attention).
- V is staged through the collective already padded with the softmax-ones
  column (65 cols/head), so gathered V tiles are matmul-ready.
- Biases are applied on the ACT engine (Identity+bias), off the DVE.
- lm_head: batched slab/output DMAs (1 per vocab chunk), PSUM evacuation
  alternates DVE/ACT, logits emitted in bf16 (host casts to fp32).
"""

import sys

for _p in ("/opt/trn_rl_repo",):
    if _p not in sys.path:
        sys.path.insert(0, _p)

import numpy as np
import ml_dtypes

import concourse.bass as bass
import concourse.bacc as bacc
import concourse.mybir as mybir
import concourse.tile as tile
from concourse.masks import make_identity

BF16 = mybir.dt.bfloat16
F32 = mybir.dt.float32
I32 = mybir.dt.int32
AF = mybir.ActivationFunctionType
ALU = mybir.AluOpType

NCORES = 8
H = 12          # heads
HD = 64         # head dim
D = 768
D3 = 3 * D      # 2304
DF = 4 * D      # 3072
KD = D // 128   # 6 d-tiles
EPS = 1e-5

bf16 = ml_dtypes.bfloat16


def build_nc(nb, L, V, stop_at=None):
    """Build the SPMD Bass module. nb = 128-token tiles per (core, batch).
    Full size: nb=2 -> 512 tokens/core, T = 8*128*nb = 2048."""
    assert nb == 2, "v2 kernel is specialized to nb=2 (512 tokens/core)"
    NT = 2 * nb          # token tiles per core (4)
    PT = NT * 128        # tokens per core (512)
    TB = nb * 128        # tokens per batch per core (256)
    NVC = (V + 511) // 512  # vocab chunks for lm_head
    VW = 65              # V cols per head incl. ones column
    VH = 6 * VW          # V cols per half (390)
    VA = H * VW          # V cols total (780)

    nc = bacc.Bacc("TRN2", target_bir_lowering=False, num_devices=NCORES)

    # ---- I/O ----
    idxs = nc.dram_tensor("idxs", [128, NT], I32, kind="ExternalInput")
    posT = nc.dram_tensor("posT", [D, PT], BF16, kind="ExternalInput")
    masks = nc.dram_tensor("masks", [128, NCORES * 128], BF16, kind="ExternalInput")
    toke = nc.dram_tensor("toke", [V, D], F32, kind="ExternalInput")
    embT = nc.dram_tensor("embT", [D, V], BF16, kind="ExternalInput")
    wqkv = [nc.dram_tensor(f"wqkv{l}", [D, D3], BF16, kind="ExternalInput") for l in range(L)]
    bqkv = [nc.dram_tensor(f"bqkv{l}", [128, 12], F32, kind="ExternalInput") for l in range(L)]
    bqv = [nc.dram_tensor(f"bqv{l}", [1, D], F32, kind="ExternalInput") for l in range(L)]
    wout = [nc.dram_tensor(f"wout{l}", [D, D], BF16, kind="ExternalInput") for l in range(L)]
    w1 = [nc.dram_tensor(f"w1_{l}", [D, DF], BF16, kind="ExternalInput") for l in range(L)]
    b1 = [nc.dram_tensor(f"b1_{l}", [128, 24], F32, kind="ExternalInput") for l in range(L)]
    w2 = [nc.dram_tensor(f"w2_{l}", [DF, D], BF16, kind="ExternalInput") for l in range(L)]
    logits = nc.dram_tensor("logits", [PT, V], BF16, kind="ExternalOutput")

    from contextlib import ExitStack
    with tile.TileContext(nc) as tc, ExitStack() as ctx:
        def pool(**kw):
            return ctx.enter_context(tc.tile_pool(**kw))
        # ---- pools ----
        const = pool(name="const", bufs=1)
        resid = pool(name="resid", bufs=1)
        acts = pool(name="acts", bufs=1)
        kvres = pool(name="kvres", bufs=1)
        wpool = pool(name="wpool", bufs=1)
        wopool = pool(name="wopool", bufs=1)
        biasp = pool(name="biasp", bufs=2)
        rot = pool(name="rot", bufs=2)
        esp = pool(name="esp", bufs=4)
        gp = pool(name="gp", bufs=4)
        w2p = pool(name="w2p", bufs=4)
        embp = pool(name="embp", bufs=2)
        logp = pool(name="logp", bufs=2)
        rowp = pool(name="rowp", bufs=4)
        psA = pool(name="psA", bufs=3, space="PSUM")   # [128,1024] 2-bank slots
        psY = pool(name="psY", bufs=2, space="PSUM")   # [128,512] 1-bank slots
        dram = pool(name="dram", bufs=2, space="DRAM")

        def psa():
            return psA.tile([128, 1024], F32, name="sa", tag="s2")

        def psy():
            return psY.tile([128, PT], F32, name="sy", tag="y")

        # ---- constants ----
        ident = const.tile([128, 128], F32, name="ident", tag="ident")
        make_identity(nc, ident)
        ones_col = const.tile([128, 1], BF16, name="ones_col", tag="ones_col")
        nc.gpsimd.memset(ones_col[:, :], 1.0)
        ones_row = const.tile([1, 128], F32, name="ones_row", tag="ones_row")
        nc.gpsimd.memset(ones_row[:, :], 1.0)
        eps_t = const.tile([1, 1], F32, name="eps_t", tag="eps_t")
        nc.gpsimd.memset(eps_t[:, :], EPS)
        zero_col = const.tile([128, 1], F32, name="zero_col", tag="zero_col")
        nc.gpsimd.memset(zero_col[:, :], 0.0)
        mask_sb = const.tile([128, NCORES * 128], BF16, name="mask_sb", tag="mask_sb")
        nc.sync.dma_start(out=mask_sb[:, :], in_=masks[:, :])
        idx_sb = const.tile([128, NT], I32, name="idx_sb", tag="idx_sb")
        nc.sync.dma_start(out=idx_sb[:, :], in_=idxs[:, :])

        # ---- persistent per-layer state ----
        xT = [resid.tile([128, PT], F32, name=f"xt{d}", tag=f"xt{d}") for d in range(KD)]
        hT = [acts.tile([128, PT], BF16, name=f"ht{d}", tag=f"ht{d}") for d in range(KD)]
        qT = [acts.tile([128, PT], BF16, name=f"qt{d}", tag=f"qt{d}") for d in range(KD)]
        yT = [acts.tile([128, PT], BF16, name=f"yt{d}", tag=f"yt{d}") for d in range(KD)]
        # gathered K^T per source core: [128, 3 r-tiles, PT] covering 384 rows (half)
        ktg = [kvres.tile([128, 3 * PT], BF16, name=f"kt{c}", tag=f"kt{c}")
               for c in range(NCORES)]
        # gathered V per source core: [128, NT key tiles, 390] (half: 6 heads x 65)
        vg = [kvres.tile([128, NT * VH], BF16, name=f"v{c}", tag=f"v{c}")
              for c in range(NCORES)]

        wbig = [wpool.tile([128, DF], BF16, name=f"wb{d}", tag=f"wb{d}") for d in range(KD)]
        w768 = [wopool.tile([128, D], BF16, name=f"w7{i}", tag=f"w7{i}") for i in range(KD)]

        def layernorm_T(dst_bf16):
            """dst[d] <- normalize(xT) across the D (partition-tiled) axis."""
            s12 = psa()  # bank0: sum, bank1: sum of squares
            s1 = s12[0:1, 0:PT]
            s2 = s12[0:1, 512:512 + PT]
            for d in range(KD):
                xb = rot.tile([128, PT], BF16, name="xb", tag="xb")
                nc.vector.tensor_copy(out=xb[:, :], in_=xT[d][:, :])
                sq = rot.tile([128, PT], BF16, name="sq", tag="sq")
                nc.vector.tensor_mul(out=sq[:, :], in0=xb[:, :], in1=xb[:, :])
                nc.tensor.matmul(out=s1, lhsT=ones_col[:, :], rhs=xb[:, :],
                                 start=(d == 0), stop=(d == KD - 1))
                nc.tensor.matmul(out=s2, lhsT=ones_col[:, :], rhs=sq[:, :],
                                 start=(d == 0), stop=(d == KD - 1))
            mrow = rowp.tile([1, PT], F32, name="mrow", tag="row")
            nc.vector.tensor_scalar(out=mrow[:, :], in0=s1, scalar1=1.0 / D,
                                    scalar2=None, op0=ALU.mult)
            vrow = rowp.tile([1, PT], F32, name="vrow", tag="row")
            nc.vector.tensor_scalar(out=vrow[:, :], in0=s2, scalar1=1.0 / D,
                                    scalar2=None, op0=ALU.mult)
            msq = rowp.tile([1, PT], F32, name="msq", tag="row")
            nc.vector.tensor_mul(out=msq[:, :], in0=mrow[:, :], in1=mrow[:, :])
            nc.vector.tensor_sub(out=vrow[:, :], in0=vrow[:, :], in1=msq[:, :])
            srow = rowp.tile([1, PT], F32, name="srow", tag="row")
            nc.scalar.activation(out=srow[:, :], in_=vrow[:, :], func=AF.Sqrt,
                                 bias=eps_t[:, :])
            rrow = rowp.tile([1, PT], F32, name="rrow", tag="row")
            nc.vector.reciprocal(out=rrow[:, :], in_=srow[:, :])
            mr = rowp.tile([1, PT], F32, name="mr", tag="row")
            nc.vector.tensor_mul(out=mr[:, :], in0=mrow[:, :], in1=rrow[:, :])
            # broadcast [1, PT] rows across 128 partitions via K=1 matmul
            bcpair = psa()
            bc_r = bcpair[:, 0:PT]
            bc_mr = bcpair[:, 512:512 + PT]
            nc.tensor.matmul(out=bc_r, lhsT=ones_row[:, :], rhs=rrow[:, :],
                             start=True, stop=True)
            nc.tensor.matmul(out=bc_mr, lhsT=ones_row[:, :], rhs=mr[:, :],
                             start=True, stop=True)
            for d in range(KD):
                t32 = rot.tile([128, PT], BF16, name="t32", tag="t32")
                nc.vector.tensor_mul(out=t32[:, :], in0=xT[d][:, :], in1=bc_r)
                nc.vector.tensor_sub(out=dst_bf16[d][:, :], in0=t32[:, :], in1=bc_mr)

        # ================= embedding =================
        # posv aliases the kbig staging buffer (same tag/shape): embed reads
        # finish before the first layer's K projection writes kbig.
        posv_t = rot.tile([128, 6 * PT], BF16, name="posv_t", tag="kbig", bufs=1)
        posv = [posv_t[:, d * PT:(d + 1) * PT] for d in range(KD)]
        for d in range(KD):
            nc.sync.dma_start(out=posv[d][:, :], in_=posT[d * 128:(d + 1) * 128, :])
        for tt in range(NT):
            xg = rot.tile([128, D], F32, name="xg", tag="xg", bufs=2)
            nc.gpsimd.indirect_dma_start(
                out=xg[:, :], out_offset=None, in_=toke[:, :],
                in_offset=bass.IndirectOffsetOnAxis(ap=idx_sb[:, tt:tt + 1], axis=0))
            for dp in range(3):  # d-pairs share a 2-bank slot
                tp = psa()
                for k in range(2):
                    d = 2 * dp + k
                    sub = tp[:, k * 512:k * 512 + 128]
                    nc.tensor.transpose(out=sub, in_=xg[:, d * 128:(d + 1) * 128],
                                        identity=ident[:, :])
                    nc.vector.tensor_tensor(
                        out=xT[d][:, tt * 128:(tt + 1) * 128], in0=sub,
                        in1=posv[d][:, tt * 128:(tt + 1) * 128], op=ALU.add)

        # ================= layers =================
        for l in range(L):
            last = l == L - 1
            def _stop(tag):
                return last and stop_at == tag
            # ---- LN1 -> hT ----
            layernorm_T(hT)
            if _stop("ln1"):
                return nc

            bq = biasp.tile([128, 12], F32, name="bq", tag="bq")
            nc.sync.dma_start(out=bq[:, :], in_=bqkv[l][:, :])
            bv = biasp.tile([1, D], F32, name="bv", tag="bv")
            nc.sync.dma_start(out=bv[:, :], in_=bqv[l][:, :])
            for d in range(KD):
                nc.sync.dma_start(out=wbig[d][:, :D3], in_=wqkv[l][d * 128:(d + 1) * 128, :])
            # V staging buffer: set the per-head softmax-ones columns now, on the
            # gpsimd queue BEFORE the AG_K trigger (whose engine-side wait would
            # otherwise delay this memset and with it the whole V phase).
            vbig = rot.tile([128, NT * VA], BF16, name="vbig", tag="vbig", bufs=1)
            vbig4 = vbig[:, :].rearrange("p (t h e) -> p t h e", t=NT, h=H)
            nc.gpsimd.memset(vbig4[:, :, :, 64:65], 1.0)

            # ---- K^T first (feeds the early AllGather) ----
            kvK_in = dram.tile([6 * 128, PT], BF16, name="kvK_in", tag="kvK_in")
            kbig = rot.tile([128, 6 * PT], BF16, name="kbig", tag="kbig", bufs=1)
            for ot in range(6, 12):
                ps = psa()
                pv = ps[:, 0:PT]
                for d in range(KD):
                    nc.tensor.matmul(out=pv, lhsT=wbig[d][:, ot * 128:(ot + 1) * 128],
                                     rhs=hT[d][:, :], start=(d == 0), stop=(d == KD - 1))
                nc.scalar.activation(out=kbig[:, (ot - 6) * PT:(ot - 5) * PT], in_=pv,
                                     func=AF.Identity, bias=bq[:, ot:ot + 1])
            nc.sync.dma_start(
                out=kvK_in[:, :].rearrange("(r p) q -> p r q", p=128),
                in_=kbig[:, :].rearrange("p (r q) -> p r q", q=PT))
            kvK_out = dram.tile([NCORES * 6 * 128, PT], BF16, name="kvK_out",
                                tag="kvK_out", addr_space="Shared")
            nc.gpsimd.collective_compute(
                "AllGather", ALU.bypass,
                replica_groups=[list(range(NCORES))],
                ins=[kvK_in[:, :].opt()], outs=[kvK_out[:, :].opt()])

            # ---- Q^T ----
            for ot in range(6):
                ps = psa()
                pv = ps[:, 0:PT]
                for d in range(KD):
                    nc.tensor.matmul(out=pv, lhsT=wbig[d][:, ot * 128:(ot + 1) * 128],
                                     rhs=hT[d][:, :], start=(d == 0), stop=(d == KD - 1))
                nc.scalar.activation(out=qT[ot][:, :], in_=pv,
                                     func=AF.Identity, bias=bq[:, ot:ot + 1])

            # ---- V (natural layout, padded with ones col per head) ----
            bvb = rot.tile([128, D], F32, name="bvb", tag="bvb", bufs=1)
            for vh in range(2):
                bcv = psy()
                nc.tensor.matmul(out=bcv[:, 0:384], lhsT=ones_row[:, :],
                                 rhs=bv[:, vh * 384:(vh + 1) * 384], start=True, stop=True)
                nc.vector.tensor_copy(out=bvb[:, vh * 384:(vh + 1) * 384],
                                      in_=bcv[:, 0:384])
            for tt in range(NT):
                for vh in range(2):
                    ps = psy()
                    for d in range(KD):
                        nc.tensor.matmul(
                            out=ps[:, 0:384],
                            lhsT=hT[d][:, tt * 128:(tt + 1) * 128],
                            rhs=wbig[d][:, D3 - D + vh * 384: D3 - D + (vh + 1) * 384],
                            start=(d == 0), stop=(d == KD - 1))
                    nc.vector.tensor_tensor(
                        out=vbig4[:, tt, vh * 6:(vh + 1) * 6, 0:64],
                        in0=ps[:, 0:384].rearrange("p (h e) -> p h e", e=64),
                        in1=bvb[:, vh * 384:(vh + 1) * 384].rearrange("p (h e) -> p h e", e=64),
                        op=ALU.add)
            kvV_in = dram.tile([PT, VA], BF16, name="kvV_in", tag="kvV_in")
            nc.sync.dma_start(
                out=kvV_in[:, :].rearrange("(t p) e -> p t e", p=128),
                in_=vbig[:, :].rearrange("p (t e) -> p t e", e=VA))
            kvV_out = dram.tile([NCORES * PT, VA], BF16, name="kvV_out",
                                tag="kvV_out", addr_space="Shared")
            nc.gpsimd.collective_compute(
                "AllGather", ALU.bypass,
                replica_groups=[list(range(NCORES))],
                ins=[kvV_in[:, :].opt()], outs=[kvV_out[:, :].opt()])
            if _stop("qkv"):
                return nc

            # prefetch next-phase weights under attention
            for k in range(KD):
                nc.sync.dma_start(out=w768[k][:, :], in_=wout[l][k * 128:(k + 1) * 128, :])
            for d in range(KD):
                nc.sync.dma_start(out=wbig[d][:, :], in_=w1[l][d * 128:(d + 1) * 128, :])
            bft = biasp.tile([128, 24], F32, name="bft", tag="bft")
            nc.sync.dma_start(out=bft[:, :], in_=b1[l][:, :])

            # ---- attention, half the heads at a time ----
            for half in range(2):
                for c in range(NCORES):
                    nc.sync.dma_start(
                        out=ktg[c][:, :].rearrange("p (r q) -> p r q", q=PT),
                        in_=kvK_out[c * 6 * 128 + half * 384:
                                    c * 6 * 128 + half * 384 + 384, :]
                        .rearrange("(r p) q -> p r q", p=128))
                    nc.sync.dma_start(
                        out=vg[c][:, :].rearrange("p (t e) -> p t e", e=VH),
                        in_=kvV_out[c * PT:(c + 1) * PT,
                                    half * VH:(half + 1) * VH]
                        .rearrange("(t p) e -> p t e", p=128))
                for pr in range(3):
                    h0 = half * 6 + 2 * pr
                    qtile = qT[h0 // 2]
                    kr = (2 * pr * 64) // 128   # r-tile within the half (= pr*... )
                    y_ps = [psy() for _ in range(2)]  # per head in pair
                    for c in range(NCORES):
                        S = [psa() for _ in range(2)]
                        for b in range(2):
                            for j in range(2):
                                for hh in range(2):
                                    kp = hh * 64
                                    nc.tensor.matmul(
                                        out=S[hh][:, b * 512 + j * 256:
                                                  b * 512 + j * 256 + 256 - j * 128],
                                        lhsT=ktg[c][kp:kp + 64,
                                                    kr * PT + (2 * b + j) * 128:
                                                    kr * PT + (2 * b + j + 1) * 128],
                                        rhs=qtile[kp:kp + 64,
                                                  b * TB + j * 128:(b + 1) * TB],
                                        start=(j == 0), stop=(j == 1))
                        for hh in range(2):
                            es = esp.tile([128, 2 * 384], BF16, name="es", tag="es")
                            nc.scalar.activation(
                                out=es[:, :].rearrange("p (b q) -> p b q", b=2),
                                in_=S[hh][:, :].rearrange("p (b q) -> p b q", b=2)[:, :, 0:384],
                                func=AF.Exp, bias=zero_col[:, :], scale=0.125)
                            es4 = es[:, :].rearrange("p (b j q) -> p b j q", b=2, q=128)
                            nc.gpsimd.tensor_tensor(
                                out=es4[:, :, 0::2, :], in0=es4[:, :, 0::2, :],
                                in1=mask_sb[:, c * 128:(c + 1) * 128]
                                .rearrange("p (b j q) -> p b j q", b=1, j=1)
                                .broadcast_to([128, 2, 2, 128]),
                                op=ALU.mult)
                            hs = 2 * pr + hh
                            es2 = es[:, :].rearrange("p (b q) -> p b q", b=2)
                            for b in range(2):
                                first = c == 0 and b == 0
                                nc.tensor.matmul(
                                    out=y_ps[hh][0:VW, b * TB:(b + 1) * TB],
                                    lhsT=vg[c][:, (2 * b) * VH + hs * VW:
                                               (2 * b) * VH + (hs + 1) * VW],
                                    rhs=es2[:, b, 0:256],
                                    start=first, stop=False)
                                lastmm = c == NCORES - 1 and b == 1
                                nc.tensor.matmul(
                                    out=y_ps[hh][0:VW, b * TB + 128:(b + 1) * TB],
                                    lhsT=vg[c][:, (2 * b + 1) * VH + hs * VW:
                                               (2 * b + 1) * VH + (hs + 1) * VW],
                                    rhs=es2[:, b, 256:384],
                                    start=False, stop=lastmm)
                    # normalize pair
                    for hh in range(2):
                        h = h0 + hh
                        zrec = rowp.tile([1, PT], F32, name="zrec", tag="row")
                        nc.vector.reciprocal(out=zrec[:, :], in_=y_ps[hh][64:65, :])
                        bcb = psa()
                        nc.tensor.matmul(out=bcb[0:64, 0:PT], lhsT=ones_row[:, 0:64],
                                         rhs=zrec[:, :], start=True, stop=True)
                        bcs = rot.tile([64, PT], BF16, name="bcs", tag="bcs")
                        nc.vector.tensor_copy(out=bcs[:, :], in_=bcb[0:64, 0:PT])
                        nc.vector.tensor_tensor(
                            out=yT[h // 2][(h % 2) * 64:(h % 2) * 64 + 64, :],
                            in0=y_ps[hh][0:64, :], in1=bcs[:, :], op=ALU.mult)

            if _stop("attn"):
                return nc
            # ---- out_proj + residual: xT += Wout^T y^T ----
            oslot = [psa() for _ in range(3)]
            oacc = [oslot[o // 2][:, (o % 2) * 512:(o % 2) * 512 + PT] for o in range(KD)]
            for k in range(KD):
                for o in range(KD):
                    nc.tensor.matmul(out=oacc[o], lhsT=w768[k][:, o * 128:(o + 1) * 128],
                                     rhs=yT[k][:, :], start=(k == 0), stop=(k == KD - 1))
            for o in range(KD):
                nc.vector.tensor_add(out=xT[o][:, :], in0=xT[o][:, :], in1=oacc[o])
            if _stop("proj"):
                return nc

            # ---- LN2 -> hT ----
            layernorm_T(hT)

            # ---- FF: g^T tile-by-tile, immediately consumed into ff2 accumulators ----
            fslot = [psa() for _ in range(3)]
            facc = [fslot[o // 2][:, (o % 2) * 512:(o % 2) * 512 + PT] for o in range(KD)]
            for ot in range(24):
                ps = psy()
                for d in range(KD):
                    nc.tensor.matmul(out=ps[:, :], lhsT=wbig[d][:, ot * 128:(ot + 1) * 128],
                                     rhs=hT[d][:, :], start=(d == 0), stop=(d == KD - 1))
                g = gp.tile([128, PT], BF16, name="g", tag="g")
                nc.scalar.activation(out=g[:, :], in_=ps[:, :], func=AF.Gelu,
                                     bias=bft[:, ot:ot + 1], scale=1.0)
                wslab = w2p.tile([128, D], BF16, name="w2s", tag="w2s")
                nc.sync.dma_start(out=wslab[:, :], in_=w2[l][ot * 128:(ot + 1) * 128, :])
                for o in range(KD):
                    nc.tensor.matmul(out=facc[o], lhsT=wslab[:, o * 128:(o + 1) * 128],
                                     rhs=g[:, :], start=(ot == 0), stop=(ot == 23))
            for o in range(KD):
                nc.vector.tensor_add(out=xT[o][:, :], in0=xT[o][:, :], in1=facc[o])

        # ================= final LN + lm_head =================
        layernorm_T(hT)
        for vc in range(NVC):
            nv = min(512, V - vc * 512)
            esl = embp.tile([128, KD * 512], BF16, name="esl", tag="esl")
            nc.sync.dma_start(
                out=esl[:, :].rearrange("p (d v) -> p d v", v=512)[:, :, 0:nv],
                in_=embT[:, vc * 512:vc * 512 + nv]
                .rearrange("(d p) v -> p d v", p=128))
            esl3 = esl[:, :].rearrange("p (d v) -> p d v", v=512)
            lsb = logp.tile([128, NT * 512], BF16, name="lsb", tag="lsb")
            lsb3 = lsb[:, :].rearrange("p (t v) -> p t v", v=512)
            for tp2 in range(2):
                ps = psa()
                for k in range(2):
                    tt = 2 * tp2 + k
                    pv = ps[:, k * 512:k * 512 + nv]
                    for d in range(KD):
                        nc.tensor.matmul(out=pv,
                                         lhsT=hT[d][:, tt * 128:(tt + 1) * 128],
                                         rhs=esl3[:, d, 0:nv], start=(d == 0),
                                         stop=(d == KD - 1))
                    if tt % 2 == 0:
                        nc.vector.tensor_copy(out=lsb3[:, tt, 0:nv], in_=pv)
                    else:
                        nc.scalar.activation(out=lsb3[:, tt, 0:nv], in_=pv,
                                             func=AF.Identity, bias=zero_col[:, :])
            nc.sync.dma_start(
                out=logits[:, vc * 512:vc * 512 + nv]
                .rearrange("(t p) v -> p t v", p=128),
                in_=lsb3[:, :, 0:nv])
    nc.finalize()
    return nc


# ------------------------------------------------------------------
# host side
# ------------------------------------------------------------------

def _prep_inputs(nb, L, V, idx, tok_emb, pos_emb, ln1_w, ln1_b, qkv_w, out_w,
                 ln2_w, ln2_b, ff1_w, ff2_w, lnf_w, lnf_b):
    NT = 2 * nb
    PT = NT * 128
    idx = np.asarray(idx).astype(np.int32)
    f = np.asarray

    shared = {
        "toke": f(tok_emb, dtype=np.float32),
        "embT": np.ascontiguousarray((f(tok_emb, dtype=np.float32) * f(lnf_w, dtype=np.float32)[None, :]).T).astype(bf16),
    }
    for l in range(L):
        wq = f(qkv_w[l], dtype=np.float32) * f(ln1_w[l], dtype=np.float32)[:, None]
        bq_full = f(ln1_b[l], dtype=np.float32) @ f(qkv_w[l], dtype=np.float32)  # [3D]
        shared[f"wqkv{l}"] = wq.astype(bf16)
        shared[f"bqkv{l}"] = np.ascontiguousarray(bq_full[:12 * 128].reshape(12, 128).T).astype(np.float32)
        shared[f"bqv{l}"] = bq_full[2 * D:].reshape(1, D).astype(np.float32)
        shared[f"wout{l}"] = f(out_w[l], dtype=np.float32).astype(bf16)
        w1e = f(ff1_w[l], dtype=np.float32) * f(ln2_w[l], dtype=np.float32)[:, None]
        b1_full = f(ln2_b[l], dtype=np.float32) @ f(ff1_w[l], dtype=np.float32)  # [4D]
        shared[f"w1_{l}"] = w1e.astype(bf16)
        shared[f"b1_{l}"] = np.ascontiguousarray(b1_full.reshape(24, 128).T).astype(np.float32)
        shared[f"w2_{l}"] = f(ff2_w[l], dtype=np.float32).astype(bf16)

    pos_f = f(pos_emb, dtype=np.float32)
    in_maps = []
    for c in range(NCORES):
        m = dict(shared)
        L_loc = np.arange(PT)
        b_loc = L_loc // (nb * 128)
        t_loc = 8 * (L_loc % (nb * 128)) + c
        idx_core = idx[b_loc, t_loc]  # [PT]
        m["idxs"] = np.ascontiguousarray(idx_core.reshape(NT, 128).T).astype(np.int32)
        m["posT"] = np.ascontiguousarray(pos_f[t_loc].T).astype(bf16)
        mk = np.zeros((128, NCORES * 128), dtype=np.float32)
        for cp in range(NCORES):
            mk[:, cp * 128:(cp + 1) * 128] = np.triu(np.ones((128, 128), np.float32),
                                                     0 if cp <= c else 1)
        m["masks"] = mk.astype(bf16)
        in_maps.append(m)
    return in_maps


_NC_CACHE = {}


def _get_nc(nb, L, V):
    key = (nb, L, V)
    if key not in _NC_CACHE:
        _NC_CACHE[key] = build_nc(nb, L, V)
    return _NC_CACHE[key]


def run_on_hw(nb, L, V, inputs, trace=False):
    from concourse import bass_utils
    nc = _get_nc(nb, L, V)
    in_maps = _prep_inputs(nb, L, V, **inputs)
    res = bass_utils.run_bass_kernel_spmd(nc, in_maps, core_ids=list(range(NCORES)),
                                          trace=trace)
    return res


def assemble(nb, L, V, results, lnf_b, tok_emb):
    T = 8 * nb * 128
    out = np.empty((2, T, V), dtype=np.float32)
    for c in range(NCORES):
        lg = results[c]["logits"].astype(np.float32).reshape(2, nb * 128, V)
        out[:, c::8, :] = lg
    lnf_b = np.asarray(lnf_b, dtype=np.float32)
    if np.any(lnf_b):
        out += (lnf_b @ np.asarray(tok_emb, dtype=np.float32).T)[None, None, :]
    return out


def kernel(**inputs):
    nb, L, V = 2, 6, 32000
    res = run_on_hw(nb, L, V, inputs)
    return assemble(nb, L, V, res.results, inputs["lnf_b"], inputs["tok_emb"])


# revision 24
# speedup vs baseline: 1.3343x; 1.0061x over previous
"""GPT-style dense transformer on 8 Trainium2 NeuronCores (v2).

Sharding: batch-split token-parallel. Cores 0-3 own batch 0, cores 4-7 own
batch 1; within its batch, core c owns positions t = 4*i + (c%4) (512 tokens
per core, all from one batch). All per-token work (LN, qkv, out_proj, ff,
lm_head) is local; attention needs all keys of the SAME batch only, so K^T
and V are AllGathered within each 4-core group once per layer (one merged
collective). The stride-4 assignment makes every core's causal structure
identical (block-lower-triangular over local key tiles, with a per-source-
core diagonal mask), so one SPMD program serves all cores.

v3 perf structure:
- Attention scores are computed per HEAD PAIR: heads (2m, 2m+1) occupy PE
  rows 0:64 / 64:128 (K=64 each). Adjacent issue with disjoint row groups
  lets the PE run both concurrently (~2x on score matmuls).
- Per (head, src-core): 4 key tiles vs the query suffix -> score matmuls of
  N=512/384/256/128. They pack into 1.5 PSUM 2-bank slots laid out so ONE
  contiguous [128, 1024] exp + one [128, 256] exp cover everything.
- Causal diagonal masks are applied multiplicatively on the GPSIMD engine
  (otherwise idle), two strided [128, 2, 128] multiplies per (head, core).
- ONE merged K+V AllGather per layer (4-rank groups [[0..3],[4..7]]); the
  Q projection overlaps its wall latency.
- V is staged through the collective already padded with the softmax-ones
  column (65 cols/head), so gathered V tiles are matmul-ready.
- Biases are applied on the ACT engine (Identity+bias), off the DVE.
- lm_head: batched slab/output DMAs (1 per vocab chunk), PSUM evacuation
  alternates DVE/ACT, logits emitted in bf16 (host casts to fp32).
"""

import sys

for _p in ("/opt/trn_rl_repo",):
    if _p not in sys.path:
        sys.path.insert(0, _p)

import numpy as np
import ml_dtypes

import concourse.bass as bass
import concourse.bacc as bacc
import concourse.mybir as mybir
import concourse.tile as tile
from concourse.masks import make_identity

BF16 = mybir.dt.bfloat16
F32 = mybir.dt.float32
I32 = mybir.dt.int32
AF = mybir.ActivationFunctionType
ALU = mybir.AluOpType

NCORES = 8
H = 12          # heads
HD = 64         # head dim
D = 768
D3 = 3 * D      # 2304
DF = 4 * D      # 3072
KD = D // 128   # 6 d-tiles
EPS = 1e-5

bf16 = ml_dtypes.bfloat16


def build_nc(nb, L, V, stop_at=None):
    """Build the SPMD Bass module. nb = 128-token tiles per (core, batch).
    Full size: nb=2 -> 512 tokens/core, T = 8*128*nb = 2048."""
    assert nb == 2, "v3 kernel is specialized to nb=2 (512 tokens/core)"
    NT = 2 * nb          # token/key tiles per core (4)
    PT = NT * 128        # tokens per core (512)
    NCG = 4              # cores per replica group (one batch per group)
    NVC = (V + 511) // 512  # vocab chunks for lm_head
    VW = 65              # V cols per head incl. ones column
    VH = 6 * VW          # V cols per half (390)
    VA = H * VW          # V cols total (780)
    KTOT = 6 * 128 * PT          # K^T elems per rank (768*512)
    VTOT = PT * VA               # V elems per rank (512*780)
    TOT = KTOT + VTOT            # merged kv elems per rank

    nc = bacc.Bacc("TRN2", target_bir_lowering=False, num_devices=NCORES)

    # ---- I/O ----
    idxs = nc.dram_tensor("idxs", [128, NT], I32, kind="ExternalInput")
    posT = nc.dram_tensor("posT", [D, PT], BF16, kind="ExternalInput")
    masks = nc.dram_tensor("masks", [128, NCG * 128], BF16, kind="ExternalInput")
    toke = nc.dram_tensor("toke", [V, D], F32, kind="ExternalInput")
    embT = nc.dram_tensor("embT", [D, V], BF16, kind="ExternalInput")
    wqkv = [nc.dram_tensor(f"wqkv{l}", [D, D3], BF16, kind="ExternalInput") for l in range(L)]
    bqkv = [nc.dram_tensor(f"bqkv{l}", [128, 12], F32, kind="ExternalInput") for l in range(L)]
    bqv = [nc.dram_tensor(f"bqv{l}", [1, D], F32, kind="ExternalInput") for l in range(L)]
    wout = [nc.dram_tensor(f"wout{l}", [D, D], BF16, kind="ExternalInput") for l in range(L)]
    w1 = [nc.dram_tensor(f"w1_{l}", [D, DF], BF16, kind="ExternalInput") for l in range(L)]
    b1 = [nc.dram_tensor(f"b1_{l}", [128, 24], F32, kind="ExternalInput") for l in range(L)]
    w2 = [nc.dram_tensor(f"w2_{l}", [DF, D], BF16, kind="ExternalInput") for l in range(L)]
    logits = nc.dram_tensor("logits", [PT, V], BF16, kind="ExternalOutput")

    from contextlib import ExitStack
    with tile.TileContext(nc) as tc, ExitStack() as ctx:
        def pool(**kw):
            return ctx.enter_context(tc.tile_pool(**kw))
        # ---- pools ----
        const = pool(name="const", bufs=1)
        resid = pool(name="resid", bufs=1)
        acts = pool(name="acts", bufs=1)
        kvres = pool(name="kvres", bufs=1)
        wpool = pool(name="wpool", bufs=1)
        wopool = pool(name="wopool", bufs=1)
        biasp = pool(name="biasp", bufs=2)
        rot = pool(name="rot", bufs=2)
        esp = pool(name="esp", bufs=4)
        gp = pool(name="gp", bufs=4)
        w2p = pool(name="w2p", bufs=4)
        embp = pool(name="embp", bufs=2)
        logp = pool(name="logp", bufs=2)
        rowp = pool(name="rowp", bufs=4)
        psA = pool(name="psA", bufs=3, space="PSUM")   # [128,1024] 2-bank slots
        psY = pool(name="psY", bufs=2, space="PSUM")   # [128,512] 1-bank slots
        dram = pool(name="dram", bufs=2, space="DRAM")

        def psa():
            return psA.tile([128, 1024], F32, name="sa", tag="s2")

        def psy():
            return psY.tile([128, PT], F32, name="sy", tag="y")

        # ---- constants ----
        ident = const.tile([128, 128], F32, name="ident", tag="ident")
        make_identity(nc, ident)
        ones_col = const.tile([128, 1], BF16, name="ones_col", tag="ones_col")
        nc.gpsimd.memset(ones_col[:, :], 1.0)
        ones_row = const.tile([1, 128], F32, name="ones_row", tag="ones_row")
        nc.gpsimd.memset(ones_row[:, :], 1.0)
        eps_t = const.tile([1, 1], F32, name="eps_t", tag="eps_t")
        nc.gpsimd.memset(eps_t[:, :], EPS)
        zero_col = const.tile([128, 1], F32, name="zero_col", tag="zero_col")
        nc.gpsimd.memset(zero_col[:, :], 0.0)
        mask_sb = const.tile([128, NCG * 128], BF16, name="mask_sb", tag="mask_sb")
        nc.sync.dma_start(out=mask_sb[:, :], in_=masks[:, :])
        idx_sb = const.tile([128, NT], I32, name="idx_sb", tag="idx_sb")
        nc.sync.dma_start(out=idx_sb[:, :], in_=idxs[:, :])

        # ---- persistent per-layer state ----
        xT = [resid.tile([128, PT], F32, name=f"xt{d}", tag=f"xt{d}") for d in range(KD)]
        hT = [acts.tile([128, PT], BF16, name=f"ht{d}", tag=f"ht{d}") for d in range(KD)]
        qT = [acts.tile([128, PT], BF16, name=f"qt{d}", tag=f"qt{d}") for d in range(KD)]
        yT = [acts.tile([128, PT], BF16, name=f"yt{d}", tag=f"yt{d}") for d in range(KD)]
        # gathered K^T per group-peer core: [128, 3 r-tiles, PT] (384 rows = half)
        ktg = [kvres.tile([128, 3 * PT], BF16, name=f"kt{c}", tag=f"kt{c}")
               for c in range(NCG)]
        # gathered V per group-peer core: [128, NT key tiles, 390] (6 heads x 65)
        vg = [kvres.tile([128, NT * VH], BF16, name=f"v{c}", tag=f"v{c}")
              for c in range(NCG)]

        wbig = [wpool.tile([128, DF], BF16, name=f"wb{d}", tag=f"wb{d}") for d in range(KD)]
        w768 = [wopool.tile([128, D], BF16, name=f"w7{i}", tag=f"w7{i}") for i in range(KD)]

        def layernorm_T(dst_bf16):
            """dst[d] <- normalize(xT) across the D (partition-tiled) axis."""
            s12 = psa()  # bank0: sum, bank1: sum of squares
            s1 = s12[0:1, 0:PT]
            s2 = s12[0:1, 512:512 + PT]
            for d in range(KD):
                xb = rot.tile([128, PT], BF16, name="xb", tag="xb")
                nc.vector.tensor_copy(out=xb[:, :], in_=xT[d][:, :])
                sq = rot.tile([128, PT], BF16, name="sq", tag="sq")
                nc.vector.tensor_mul(out=sq[:, :], in0=xb[:, :], in1=xb[:, :])
                nc.tensor.matmul(out=s1, lhsT=ones_col[:, :], rhs=xb[:, :],
                                 start=(d == 0), stop=(d == KD - 1))
                nc.tensor.matmul(out=s2, lhsT=ones_col[:, :], rhs=sq[:, :],
                                 start=(d == 0), stop=(d == KD - 1))
            mrow = rowp.tile([1, PT], F32, name="mrow", tag="row")
            nc.vector.tensor_scalar(out=mrow[:, :], in0=s1, scalar1=1.0 / D,
                                    scalar2=None, op0=ALU.mult)
            vrow = rowp.tile([1, PT], F32, name="vrow", tag="row")
            nc.vector.tensor_scalar(out=vrow[:, :], in0=s2, scalar1=1.0 / D,
                                    scalar2=None, op0=ALU.mult)
            msq = rowp.tile([1, PT], F32, name="msq", tag="row")
            nc.vector.tensor_mul(out=msq[:, :], in0=mrow[:, :], in1=mrow[:, :])
            nc.vector.tensor_sub(out=vrow[:, :], in0=vrow[:, :], in1=msq[:, :])
            srow = rowp.tile([1, PT], F32, name="srow", tag="row")
            nc.scalar.activation(out=srow[:, :], in_=vrow[:, :], func=AF.Sqrt,
                                 bias=eps_t[:, :])
            rrow = rowp.tile([1, PT], F32, name="rrow", tag="row")
            nc.vector.reciprocal(out=rrow[:, :], in_=srow[:, :])
            mr = rowp.tile([1, PT], F32, name="mr", tag="row")
            nc.vector.tensor_mul(out=mr[:, :], in0=mrow[:, :], in1=rrow[:, :])
            # broadcast [1, PT] rows across 128 partitions via K=1 matmul
            bcpair = psa()
            bc_r = bcpair[:, 0:PT]
            bc_mr = bcpair[:, 512:512 + PT]
            nc.tensor.matmul(out=bc_r, lhsT=ones_row[:, :], rhs=rrow[:, :],
                             start=True, stop=True)
            nc.tensor.matmul(out=bc_mr, lhsT=ones_row[:, :], rhs=mr[:, :],
                             start=True, stop=True)
            for d in range(KD):
                t32 = rot.tile([128, PT], BF16, name="t32", tag="t32")
                nc.vector.tensor_mul(out=t32[:, :], in0=xT[d][:, :], in1=bc_r)
                nc.vector.tensor_sub(out=dst_bf16[d][:, :], in0=t32[:, :], in1=bc_mr)

        # ================= embedding =================
        # posv aliases the kbig staging buffer (same tag/shape): embed reads
        # finish before the first layer's K projection writes kbig.
        posv_t = rot.tile([128, 6 * PT], BF16, name="posv_t", tag="kbig", bufs=1)
        posv = [posv_t[:, d * PT:(d + 1) * PT] for d in range(KD)]
        for d in range(KD):
            nc.sync.dma_start(out=posv[d][:, :], in_=posT[d * 128:(d + 1) * 128, :])
        for tt in range(NT):
            xg = rot.tile([128, D], F32, name="xg", tag="xg", bufs=2)
            nc.gpsimd.indirect_dma_start(
                out=xg[:, :], out_offset=None, in_=toke[:, :],
                in_offset=bass.IndirectOffsetOnAxis(ap=idx_sb[:, tt:tt + 1], axis=0))
            for dp in range(3):  # d-pairs share a 2-bank slot
                tp = psa()
                for k in range(2):
                    d = 2 * dp + k
                    sub = tp[:, k * 512:k * 512 + 128]
                    nc.tensor.transpose(out=sub, in_=xg[:, d * 128:(d + 1) * 128],
                                        identity=ident[:, :])
                    nc.vector.tensor_tensor(
                        out=xT[d][:, tt * 128:(tt + 1) * 128], in0=sub,
                        in1=posv[d][:, tt * 128:(tt + 1) * 128], op=ALU.add)

        # ================= layers =================
        for l in range(L):
            last = l == L - 1
            def _stop(tag):
                return last and stop_at == tag
            # ---- LN1 -> hT ----
            layernorm_T(hT)
            if _stop("ln1"):
                return nc

            bq = biasp.tile([128, 12], F32, name="bq", tag="bq")
            nc.sync.dma_start(out=bq[:, :], in_=bqkv[l][:, :])
            bv = biasp.tile([1, D], F32, name="bv", tag="bv")
            nc.sync.dma_start(out=bv[:, :], in_=bqv[l][:, :])
            for d in range(KD):
                nc.sync.dma_start(out=wbig[d][:, :D3], in_=wqkv[l][d * 128:(d + 1) * 128, :])
            # V staging buffer: set the per-head softmax-ones columns now, on the
            # gpsimd queue BEFORE the AG_K trigger (whose engine-side wait would
            # otherwise delay this memset and with it the whole V phase).
            vbig = rot.tile([128, NT * VA], BF16, name="vbig", tag="vbig", bufs=1)
            vbig4 = vbig[:, :].rearrange("p (t h e) -> p t h e", t=NT, h=H)
            nc.gpsimd.memset(vbig4[:, :, :, 64:65], 1.0)

            # ---- K^T, then V, then one merged AllGather; Q overlaps the AG ----
            kv_in = dram.tile([1, TOT], BF16, name="kv_in", tag="kv_in")
            kvf = kv_in[:, :].rearrange("o n -> (o n)")
            kbig = rot.tile([128, 6 * PT], BF16, name="kbig", tag="kbig", bufs=1)
            for ot in range(6, 12):
                ps = psa()
                pv = ps[:, 0:PT]
                for d in range(KD):
                    nc.tensor.matmul(out=pv, lhsT=wbig[d][:, ot * 128:(ot + 1) * 128],
                                     rhs=hT[d][:, :], start=(d == 0), stop=(d == KD - 1))
                nc.scalar.activation(out=kbig[:, (ot - 6) * PT:(ot - 5) * PT], in_=pv,
                                     func=AF.Identity, bias=bq[:, ot:ot + 1])
            nc.sync.dma_start(
                out=kvf[0:KTOT].rearrange("(r p q) -> p r q", p=128, q=PT),
                in_=kbig[:, :].rearrange("p (r q) -> p r q", q=PT))

            # ---- V (natural layout, padded with ones col per head) ----
            bvb = rot.tile([128, D], F32, name="bvb", tag="bvb", bufs=1)
            for vh in range(2):
                bcv = psy()
                nc.tensor.matmul(out=bcv[:, 0:384], lhsT=ones_row[:, :],
                                 rhs=bv[:, vh * 384:(vh + 1) * 384], start=True, stop=True)
                nc.vector.tensor_copy(out=bvb[:, vh * 384:(vh + 1) * 384],
                                      in_=bcv[:, 0:384])
            for tt in range(NT):
                for vh in range(2):
                    ps = psy()
                    for d in range(KD):
                        nc.tensor.matmul(
                            out=ps[:, 0:384],
                            lhsT=hT[d][:, tt * 128:(tt + 1) * 128],
                            rhs=wbig[d][:, D3 - D + vh * 384: D3 - D + (vh + 1) * 384],
                            start=(d == 0), stop=(d == KD - 1))
                    nc.vector.tensor_tensor(
                        out=vbig4[:, tt, vh * 6:(vh + 1) * 6, 0:64],
                        in0=ps[:, 0:384].rearrange("p (h e) -> p h e", e=64),
                        in1=bvb[:, vh * 384:(vh + 1) * 384].rearrange("p (h e) -> p h e", e=64),
                        op=ALU.add)
            nc.sync.dma_start(
                out=kvf[KTOT:TOT].rearrange("(t p e) -> p t e", p=128, e=VA),
                in_=vbig[:, :].rearrange("p (t e) -> p t e", e=VA))
            kv_out = dram.tile([1, NCG * TOT], BF16, name="kv_out", tag="kv_out")
            kvof = kv_out[:, :].rearrange("o n -> (o n)")
            nc.gpsimd.collective_compute(
                "AllGather", ALU.bypass,
                replica_groups=[[0, 1, 2, 3], [4, 5, 6, 7]],
                ins=[kv_in[:, :].opt()], outs=[kv_out[:, :].opt()])

            # ---- Q^T (overlaps the AllGather) ----
            for ot in range(6):
                ps = psa()
                pv = ps[:, 0:PT]
                for d in range(KD):
                    nc.tensor.matmul(out=pv, lhsT=wbig[d][:, ot * 128:(ot + 1) * 128],
                                     rhs=hT[d][:, :], start=(d == 0), stop=(d == KD - 1))
                nc.scalar.activation(out=qT[ot][:, :], in_=pv,
                                     func=AF.Identity, bias=bq[:, ot:ot + 1])
            if _stop("qkv"):
                return nc

            # prefetch next-phase weights under attention
            for k in range(KD):
                nc.sync.dma_start(out=w768[k][:, :], in_=wout[l][k * 128:(k + 1) * 128, :])
            for d in range(KD):
                nc.sync.dma_start(out=wbig[d][:, :], in_=w1[l][d * 128:(d + 1) * 128, :])
            bft = biasp.tile([128, 24], F32, name="bft", tag="bft")
            nc.sync.dma_start(out=bft[:, :], in_=b1[l][:, :])

            # ---- attention, half the heads at a time ----
            # es layout per (head, src-core): [t0 512 | t1 384 | t3 128 | t2 256]
            # so one contiguous [128,1024] exp covers t0/t1/t3 (from the 2-bank
            # score slot) and a second [128,256] exp covers t2.
            for half in range(2):
                for c in range(NCG):
                    nc.sync.dma_start(
                        out=ktg[c][:, :].rearrange("p (r q) -> p r q", q=PT),
                        in_=kvof[c * TOT + half * 3 * 128 * PT:
                                 c * TOT + (half * 3 + 3) * 128 * PT]
                        .rearrange("(r p q) -> p r q", p=128, q=PT))
                    nc.sync.dma_start(
                        out=vg[c][:, :].rearrange("p (t e) -> p t e", e=VH),
                        in_=kvof[c * TOT + KTOT: (c + 1) * TOT]
                        .rearrange("(t p e) -> p t e", p=128, e=VA)
                        [:, :, half * VH:(half + 1) * VH])
                for pr in range(3):
                    h0 = half * 6 + 2 * pr
                    qtile = qT[h0 // 2]
                    y_ps = [psy() for _ in range(2)]  # per head in pair
                    for c in range(NCG):
                        SA = [psa() for _ in range(2)]   # per head: t0 | t1,t3
                        SB = psa()                       # shared: h0-t2 | h1-t2
                        # (slot, col offset, start, stop) per key tile, in
                        # emission order t0, t1, t3, t2
                        for t, off, n, st, sp in ((0, 0, 512, True, True),
                                                  (1, 512, 384, True, False),
                                                  (3, 896, 128, False, True)):
                            for hh in range(2):
                                kp = hh * 64
                                nc.tensor.matmul(
                                    out=SA[hh][:, off:off + n],
                                    lhsT=ktg[c][kp:kp + 64,
                                                pr * PT + t * 128:pr * PT + (t + 1) * 128],
                                    rhs=qtile[kp:kp + 64, t * 128:PT],
                                    start=st, stop=sp)
                        for hh in range(2):
                            kp = hh * 64
                            nc.tensor.matmul(
                                out=SB[:, hh * 512:hh * 512 + 256],
                                lhsT=ktg[c][kp:kp + 64,
                                            pr * PT + 2 * 128:pr * PT + 3 * 128],
                                rhs=qtile[kp:kp + 64, 2 * 128:PT],
                                start=True, stop=True)
                        for hh in range(2):
                            es = esp.tile([128, 1280], BF16, name="es", tag="es")
                            nc.scalar.activation(
                                out=es[:, 0:1024], in_=SA[hh][:, 0:1024],
                                func=AF.Exp, bias=zero_col[:, :], scale=0.125)
                            nc.scalar.activation(
                                out=es[:, 1024:1280],
                                in_=SB[:, hh * 512:hh * 512 + 256],
                                func=AF.Exp, bias=zero_col[:, :], scale=0.125)
                            # diagonal masks: t0 at 0, t1 at 512, t3 at 896,
                            # t2 at 1024 -> two strided [128, 2, 128] muls
                            mview = (mask_sb[:, c * 128:(c + 1) * 128]
                                     .rearrange("p (a q) -> p a q", a=1)
                                     .broadcast_to([128, 2, 128]))
                            d1 = es[:, 0:1024].rearrange("p (a q) -> p a q", a=2)[:, :, 0:128]
                            nc.gpsimd.tensor_tensor(out=d1, in0=d1, in1=mview,
                                                    op=ALU.mult)
                            d2 = es[:, 896:1152].rearrange("p (a q) -> p a q", a=2)
                            nc.gpsimd.tensor_tensor(out=d2, in0=d2, in1=mview,
                                                    op=ALU.mult)
                            hs = 2 * pr + hh
                            for t, off, n in ((0, 0, 512), (1, 512, 384),
                                              (3, 896, 128), (2, 1024, 256)):
                                nc.tensor.matmul(
                                    out=y_ps[hh][0:VW, t * 128:PT],
                                    lhsT=vg[c][:, t * VH + hs * VW:
                                               t * VH + (hs + 1) * VW],
                                    rhs=es[:, off:off + n],
                                    start=(c == 0 and t == 0),
                                    stop=(c == NCG - 1 and t == 2))
                    # normalize pair
                    for hh in range(2):
                        h = h0 + hh
                        zrec = rowp.tile([1, PT], F32, name="zrec", tag="row")
                        nc.vector.reciprocal(out=zrec[:, :], in_=y_ps[hh][64:65, :])
                        bcb = psa()
                        nc.tensor.matmul(out=bcb[0:64, 0:PT], lhsT=ones_row[:, 0:64],
                                         rhs=zrec[:, :], start=True, stop=True)
                        bcs = rot.tile([64, PT], BF16, name="bcs", tag="bcs")
                        nc.vector.tensor_copy(out=bcs[:, :], in_=bcb[0:64, 0:PT])
                        nc.vector.tensor_tensor(
                            out=yT[h // 2][(h % 2) * 64:(h % 2) * 64 + 64, :],
                            in0=y_ps[hh][0:64, :], in1=bcs[:, :], op=ALU.mult)

            if _stop("attn"):
                return nc
            # ---- out_proj + residual: xT += Wout^T y^T ----
            oslot = [psa() for _ in range(3)]
            oacc = [oslot[o // 2][:, (o % 2) * 512:(o % 2) * 512 + PT] for o in range(KD)]
            for k in range(KD):
                for o in range(KD):
                    nc.tensor.matmul(out=oacc[o], lhsT=w768[k][:, o * 128:(o + 1) * 128],
                                     rhs=yT[k][:, :], start=(k == 0), stop=(k == KD - 1))
            for o in range(KD):
                nc.vector.tensor_add(out=xT[o][:, :], in0=xT[o][:, :], in1=oacc[o])
            if _stop("proj"):
                return nc

            # ---- LN2 -> hT ----
            layernorm_T(hT)

            # ---- FF: g^T tile-by-tile, immediately consumed into ff2 accumulators ----
            fslot = [psa() for _ in range(3)]
            facc = [fslot[o // 2][:, (o % 2) * 512:(o % 2) * 512 + PT] for o in range(KD)]
            for ot in range(24):
                ps = psy()
                for d in range(KD):
                    nc.tensor.matmul(out=ps[:, :], lhsT=wbig[d][:, ot * 128:(ot + 1) * 128],
                                     rhs=hT[d][:, :], start=(d == 0), stop=(d == KD - 1))
                g = gp.tile([128, PT], BF16, name="g", tag="g")
                nc.scalar.activation(out=g[:, :], in_=ps[:, :], func=AF.Gelu,
                                     bias=bft[:, ot:ot + 1], scale=1.0)
                wslab = w2p.tile([128, D], BF16, name="w2s", tag="w2s")
                nc.sync.dma_start(out=wslab[:, :], in_=w2[l][ot * 128:(ot + 1) * 128, :])
                for o in range(KD):
                    nc.tensor.matmul(out=facc[o], lhsT=wslab[:, o * 128:(o + 1) * 128],
                                     rhs=g[:, :], start=(ot == 0), stop=(ot == 23))
            for o in range(KD):
                nc.vector.tensor_add(out=xT[o][:, :], in0=xT[o][:, :], in1=facc[o])

        # ================= final LN + lm_head =================
        layernorm_T(hT)
        for vc in range(NVC):
            nv = min(512, V - vc * 512)
            esl = embp.tile([128, KD * 512], BF16, name="esl", tag="esl")
            nc.sync.dma_start(
                out=esl[:, :].rearrange("p (d v) -> p d v", v=512)[:, :, 0:nv],
                in_=embT[:, vc * 512:vc * 512 + nv]
                .rearrange("(d p) v -> p d v", p=128))
            esl3 = esl[:, :].rearrange("p (d v) -> p d v", v=512)
            lsb = logp.tile([128, NT * 512], BF16, name="lsb", tag="lsb")
            lsb3 = lsb[:, :].rearrange("p (t v) -> p t v", v=512)
            for tp2 in range(2):
                ps = psa()
                for k in range(2):
                    tt = 2 * tp2 + k
                    pv = ps[:, k * 512:k * 512 + nv]
                    for d in range(KD):
                        nc.tensor.matmul(out=pv,
                                         lhsT=hT[d][:, tt * 128:(tt + 1) * 128],
                                         rhs=esl3[:, d, 0:nv], start=(d == 0),
                                         stop=(d == KD - 1))
                    if tt % 2 == 0:
                        nc.vector.tensor_copy(out=lsb3[:, tt, 0:nv], in_=pv)
                    else:
                        nc.scalar.activation(out=lsb3[:, tt, 0:nv], in_=pv,
                                             func=AF.Identity, bias=zero_col[:, :])
            nc.sync.dma_start(
                out=logits[:, vc * 512:vc * 512 + nv]
                .rearrange("(t p) v -> p t v", p=128),
                in_=lsb3[:, :, 0:nv])
    nc.finalize()
    return nc


# ------------------------------------------------------------------
# host side
# ------------------------------------------------------------------

def _prep_inputs(nb, L, V, idx, tok_emb, pos_emb, ln1_w, ln1_b, qkv_w, out_w,
                 ln2_w, ln2_b, ff1_w, ff2_w, lnf_w, lnf_b):
    NT = 2 * nb
    PT = NT * 128
    idx = np.asarray(idx).astype(np.int32)
    f = np.asarray

    shared = {
        "toke": f(tok_emb, dtype=np.float32),
        "embT": np.ascontiguousarray((f(tok_emb, dtype=np.float32) * f(lnf_w, dtype=np.float32)[None, :]).T).astype(bf16),
    }
    for l in range(L):
        wq = f(qkv_w[l], dtype=np.float32) * f(ln1_w[l], dtype=np.float32)[:, None]
        bq_full = f(ln1_b[l], dtype=np.float32) @ f(qkv_w[l], dtype=np.float32)  # [3D]
        shared[f"wqkv{l}"] = wq.astype(bf16)
        shared[f"bqkv{l}"] = np.ascontiguousarray(bq_full[:12 * 128].reshape(12, 128).T).astype(np.float32)
        shared[f"bqv{l}"] = bq_full[2 * D:].reshape(1, D).astype(np.float32)
        shared[f"wout{l}"] = f(out_w[l], dtype=np.float32).astype(bf16)
        w1e = f(ff1_w[l], dtype=np.float32) * f(ln2_w[l], dtype=np.float32)[:, None]
        b1_full = f(ln2_b[l], dtype=np.float32) @ f(ff1_w[l], dtype=np.float32)  # [4D]
        shared[f"w1_{l}"] = w1e.astype(bf16)
        shared[f"b1_{l}"] = np.ascontiguousarray(b1_full.reshape(24, 128).T).astype(np.float32)
        shared[f"w2_{l}"] = f(ff2_w[l], dtype=np.float32).astype(bf16)

    pos_f = f(pos_emb, dtype=np.float32)
    in_maps = []
    for c in range(NCORES):
        m = dict(shared)
        b_core = c // 4          # batch owned by this core's group
        lane = c % 4             # position stride-4 lane within the batch
        t_loc = 4 * np.arange(PT) + lane
        idx_core = idx[b_core, t_loc]  # [PT]
        m["idxs"] = np.ascontiguousarray(idx_core.reshape(NT, 128).T).astype(np.int32)
        m["posT"] = np.ascontiguousarray(pos_f[t_loc].T).astype(bf16)
        mk = np.zeros((128, 4 * 128), dtype=np.float32)
        for cp in range(4):
            mk[:, cp * 128:(cp + 1) * 128] = np.triu(np.ones((128, 128), np.float32),
                                                     0 if cp <= lane else 1)
        m["masks"] = mk.astype(bf16)
        in_maps.append(m)
    return in_maps


_NC_CACHE = {}


def _get_nc(nb, L, V):
    key = (nb, L, V)
    if key not in _NC_CACHE:
        _NC_CACHE[key] = build_nc(nb, L, V)
    return _NC_CACHE[key]


def run_on_hw(nb, L, V, inputs, trace=False):
    from concourse import bass_utils
    nc = _get_nc(nb, L, V)
    in_maps = _prep_inputs(nb, L, V, **inputs)
    res = bass_utils.run_bass_kernel_spmd(nc, in_maps, core_ids=list(range(NCORES)),
                                          trace=trace)
    return res


def assemble(nb, L, V, results, lnf_b, tok_emb):
    T = 8 * nb * 128
    out = np.empty((2, T, V), dtype=np.float32)
    for c in range(NCORES):
        lg = results[c]["logits"].astype(np.float32)  # [512, V]
        out[c // 4, (c % 4)::4, :] = lg
    lnf_b = np.asarray(lnf_b, dtype=np.float32)
    if np.any(lnf_b):
        out += (lnf_b @ np.asarray(tok_emb, dtype=np.float32).T)[None, None, :]
    return out


def kernel(**inputs):
    nb, L, V = 2, 6, 32000
    res = run_on_hw(nb, L, V, inputs)
    return assemble(nb, L, V, res.results, inputs["lnf_b"], inputs["tok_emb"])


# revision 35
# speedup vs baseline: 1.3714x; 1.0278x over previous
"""GPT-style dense transformer on 8 Trainium2 NeuronCores (v2).

Sharding: batch-split token-parallel. Cores 0-3 own batch 0, cores 4-7 own
batch 1; within its batch, core c owns positions t = 4*i + (c%4) (512 tokens
per core, all from one batch). All per-token work (LN, qkv, out_proj, ff,
lm_head) is local; attention needs all keys of the SAME batch only, so K^T
and V are AllGathered within each 4-core group once per layer (one merged
collective). The stride-4 assignment makes every core's causal structure
identical (block-lower-triangular over local key tiles, with a per-source-
core diagonal mask), so one SPMD program serves all cores.

v3 perf structure:
- Attention scores are computed per HEAD PAIR: heads (2m, 2m+1) occupy PE
  rows 0:64 / 64:128 (K=64 each). Adjacent issue with disjoint row groups
  lets the PE run both concurrently (~2x on score matmuls).
- Per (head, src-core): 4 key tiles vs the query suffix -> score matmuls of
  N=512/384/256/128. They pack into 1.5 PSUM 2-bank slots laid out so ONE
  contiguous [128, 1024] exp + one [128, 256] exp cover everything.
- Causal diagonal masks are applied multiplicatively on the GPSIMD engine
  (otherwise idle), two strided [128, 2, 128] multiplies per (head, core).
- ONE merged K+V AllGather per layer (4-rank groups [[0..3],[4..7]]); the
  Q projection overlaps its wall latency.
- V is staged through the collective already padded with the softmax-ones
  column (65 cols/head), so gathered V tiles are matmul-ready.
- Biases are applied on the ACT engine (Identity+bias), off the DVE.
- lm_head: batched slab/output DMAs (1 per vocab chunk), PSUM evacuation
  alternates DVE/ACT, logits emitted in bf16 (host casts to fp32).
"""

import sys

for _p in ("/opt/trn_rl_repo",):
    if _p not in sys.path:
        sys.path.insert(0, _p)

import numpy as np
import ml_dtypes

import concourse.bass as bass
import concourse.bacc as bacc
import concourse.mybir as mybir
import concourse.tile as tile
from concourse.masks import make_identity

BF16 = mybir.dt.bfloat16
F32 = mybir.dt.float32
I32 = mybir.dt.int32
AF = mybir.ActivationFunctionType
ALU = mybir.AluOpType

NCORES = 8
H = 12          # heads
HD = 64         # head dim
D = 768
D3 = 3 * D      # 2304
DF = 4 * D      # 3072
KD = D // 128   # 6 d-tiles
EPS = 1e-5

bf16 = ml_dtypes.bfloat16


def build_nc(nb, L, V, stop_at=None):
    """Build the SPMD Bass module. nb = 128-token tiles per (core, batch).
    Full size: nb=2 -> 512 tokens/core, T = 8*128*nb = 2048."""
    assert nb == 2, "v4 kernel is specialized to nb=2 (512 tokens/core)"
    NT = 2 * nb          # token tiles per core (4)
    PT = NT * 128        # tokens per core (512)
    TB = nb * 128        # tokens per batch per core (256)
    NVC = (V + 511) // 512  # vocab chunks for lm_head
    VW = 65              # V cols per head incl. ones column
    VH = 6 * VW          # V cols per head-half (390)
    KT2 = 3 * 128 * PT           # K^T elems per rank per half (384*512)
    VT2 = PT * VH                # V elems per rank per half (512*390)
    TOT2 = KT2 + VT2             # kv elems per rank per half

    nc = bacc.Bacc("TRN2", target_bir_lowering=False, num_devices=NCORES)

    # ---- I/O ----
    idxs = nc.dram_tensor("idxs", [128, NT], I32, kind="ExternalInput")
    posT = nc.dram_tensor("posT", [D, PT], BF16, kind="ExternalInput")
    masks = nc.dram_tensor("masks", [128, NCORES * 128], BF16, kind="ExternalInput")
    toke = nc.dram_tensor("toke", [V, D], F32, kind="ExternalInput")
    embT = nc.dram_tensor("embT", [D, V], BF16, kind="ExternalInput")
    wqkv = [nc.dram_tensor(f"wqkv{l}", [D, D3], BF16, kind="ExternalInput") for l in range(L)]
    bqkv = [nc.dram_tensor(f"bqkv{l}", [128, 12], F32, kind="ExternalInput") for l in range(L)]
    bqv = [nc.dram_tensor(f"bqv{l}", [1, D], F32, kind="ExternalInput") for l in range(L)]
    wout = [nc.dram_tensor(f"wout{l}", [D, D], BF16, kind="ExternalInput") for l in range(L)]
    w1 = [nc.dram_tensor(f"w1_{l}", [D, DF], BF16, kind="ExternalInput") for l in range(L)]
    b1 = [nc.dram_tensor(f"b1_{l}", [128, 24], F32, kind="ExternalInput") for l in range(L)]
    w2 = [nc.dram_tensor(f"w2_{l}", [DF, D], BF16, kind="ExternalInput") for l in range(L)]
    logits = nc.dram_tensor("logits", [PT, V], BF16, kind="ExternalOutput")

    from contextlib import ExitStack
    with tile.TileContext(nc) as tc, ExitStack() as ctx:
        def pool(**kw):
            return ctx.enter_context(tc.tile_pool(**kw))
        # ---- pools ----
        const = pool(name="const", bufs=1)
        resid = pool(name="resid", bufs=1)
        acts = pool(name="acts", bufs=1)
        kvres = pool(name="kvres", bufs=1)
        wpool = pool(name="wpool", bufs=1)
        wopool = pool(name="wopool", bufs=1)
        biasp = pool(name="biasp", bufs=2)
        rot = pool(name="rot", bufs=2)
        esp = pool(name="esp", bufs=4)
        gp = pool(name="gp", bufs=4)
        w2p = pool(name="w2p", bufs=4)
        embp = pool(name="embp", bufs=2)
        logp = pool(name="logp", bufs=2)
        rowp = pool(name="rowp", bufs=4)
        psA = pool(name="psA", bufs=3, space="PSUM")   # [128,1024] 2-bank slots
        psY = pool(name="psY", bufs=2, space="PSUM")   # [128,512] 1-bank slots
        dram = pool(name="dram", bufs=2, space="DRAM")

        def psa():
            return psA.tile([128, 1024], F32, name="sa", tag="s2")

        def psy():
            return psY.tile([128, PT], F32, name="sy", tag="y")

        # ---- constants ----
        ident = const.tile([128, 128], F32, name="ident", tag="ident")
        make_identity(nc, ident)
        ones_col = const.tile([128, 1], BF16, name="ones_col", tag="ones_col")
        nc.gpsimd.memset(ones_col[:, :], 1.0)
        ones_row = const.tile([1, 128], F32, name="ones_row", tag="ones_row")
        nc.gpsimd.memset(ones_row[:, :], 1.0)
        eps_t = const.tile([1, 1], F32, name="eps_t", tag="eps_t")
        nc.gpsimd.memset(eps_t[:, :], EPS)
        zero_col = const.tile([128, 1], F32, name="zero_col", tag="zero_col")
        nc.gpsimd.memset(zero_col[:, :], 0.0)
        mask_sb = const.tile([128, NCORES * 128], BF16, name="mask_sb", tag="mask_sb")
        nc.sync.dma_start(out=mask_sb[:, :], in_=masks[:, :])
        idx_sb = const.tile([128, NT], I32, name="idx_sb", tag="idx_sb")
        nc.sync.dma_start(out=idx_sb[:, :], in_=idxs[:, :])

        # ---- persistent per-layer state ----
        xT = [resid.tile([128, PT], F32, name=f"xt{d}", tag=f"xt{d}") for d in range(KD)]
        hT = [acts.tile([128, PT], BF16, name=f"ht{d}", tag=f"ht{d}") for d in range(KD)]
        qT = [acts.tile([128, PT], BF16, name=f"qt{d}", tag=f"qt{d}") for d in range(KD)]
        yT = [acts.tile([128, PT], BF16, name=f"yt{d}", tag=f"yt{d}") for d in range(KD)]
        # gathered K^T per source core: [128, 3 r-tiles, PT] (384 rows = half)
        ktg = [kvres.tile([128, 3 * PT], BF16, name=f"kt{c}", tag=f"kt{c}")
               for c in range(NCORES)]
        # gathered V per source core: [128, NT key tiles, 390] (6 heads x 65)
        vg = [kvres.tile([128, NT * VH], BF16, name=f"v{c}", tag=f"v{c}")
              for c in range(NCORES)]

        wbig = [wpool.tile([128, DF], BF16, name=f"wb{d}", tag=f"wb{d}") for d in range(KD)]
        w768 = [wopool.tile([128, D], BF16, name=f"w7{i}", tag=f"w7{i}") for i in range(KD)]

        def layernorm_T(dst_bf16):
            """dst[d] <- normalize(xT) across the D (partition-tiled) axis."""
            s12 = psa()  # bank0: sum, bank1: sum of squares
            s1 = s12[0:1, 0:PT]
            s2 = s12[0:1, 512:512 + PT]
            for d in range(KD):
                xb = rot.tile([128, PT], BF16, name="xb", tag="xb")
                nc.vector.tensor_copy(out=xb[:, :], in_=xT[d][:, :])
                sq = rot.tile([128, PT], BF16, name="sq", tag="sq")
                nc.vector.tensor_mul(out=sq[:, :], in0=xb[:, :], in1=xb[:, :])
                nc.tensor.matmul(out=s1, lhsT=ones_col[:, :], rhs=xb[:, :],
                                 start=(d == 0), stop=(d == KD - 1))
                nc.tensor.matmul(out=s2, lhsT=ones_col[:, :], rhs=sq[:, :],
                                 start=(d == 0), stop=(d == KD - 1))
            mrow = rowp.tile([1, PT], F32, name="mrow", tag="row")
            nc.vector.tensor_scalar(out=mrow[:, :], in0=s1, scalar1=1.0 / D,
                                    scalar2=None, op0=ALU.mult)
            vrow = rowp.tile([1, PT], F32, name="vrow", tag="row")
            nc.vector.tensor_scalar(out=vrow[:, :], in0=s2, scalar1=1.0 / D,
                                    scalar2=None, op0=ALU.mult)
            msq = rowp.tile([1, PT], F32, name="msq", tag="row")
            nc.vector.tensor_mul(out=msq[:, :], in0=mrow[:, :], in1=mrow[:, :])
            nc.vector.tensor_sub(out=vrow[:, :], in0=vrow[:, :], in1=msq[:, :])
            srow = rowp.tile([1, PT], F32, name="srow", tag="row")
            nc.scalar.activation(out=srow[:, :], in_=vrow[:, :], func=AF.Sqrt,
                                 bias=eps_t[:, :])
            rrow = rowp.tile([1, PT], F32, name="rrow", tag="row")
            nc.vector.reciprocal(out=rrow[:, :], in_=srow[:, :])
            mr = rowp.tile([1, PT], F32, name="mr", tag="row")
            nc.vector.tensor_mul(out=mr[:, :], in0=mrow[:, :], in1=rrow[:, :])
            # broadcast [1, PT] rows across 128 partitions via K=1 matmul
            bcpair = psa()
            bc_r = bcpair[:, 0:PT]
            bc_mr = bcpair[:, 512:512 + PT]
            nc.tensor.matmul(out=bc_r, lhsT=ones_row[:, :], rhs=rrow[:, :],
                             start=True, stop=True)
            nc.tensor.matmul(out=bc_mr, lhsT=ones_row[:, :], rhs=mr[:, :],
                             start=True, stop=True)
            for d in range(KD):
                t32 = rot.tile([128, PT], BF16, name="t32", tag="t32")
                nc.vector.tensor_mul(out=t32[:, :], in0=xT[d][:, :], in1=bc_r)
                nc.vector.tensor_sub(out=dst_bf16[d][:, :], in0=t32[:, :], in1=bc_mr)

        # ================= embedding =================
        # posv aliases the K-staging buffers (same tags/shapes): embed reads
        # finish before the first layer's K projection writes them.
        posv_a = rot.tile([128, 3 * PT], BF16, name="posv_a", tag="kbig0", bufs=1)
        posv_b = rot.tile([128, 3 * PT], BF16, name="posv_b", tag="kbig1", bufs=1)
        posv = [posv_a[:, d * PT:(d + 1) * PT] for d in range(3)] + \
               [posv_b[:, d * PT:(d + 1) * PT] for d in range(3)]
        for d in range(KD):
            nc.sync.dma_start(out=posv[d][:, :], in_=posT[d * 128:(d + 1) * 128, :])
        for tt in range(NT):
            xg = rot.tile([128, D], F32, name="xg", tag="xg", bufs=2)
            nc.gpsimd.indirect_dma_start(
                out=xg[:, :], out_offset=None, in_=toke[:, :],
                in_offset=bass.IndirectOffsetOnAxis(ap=idx_sb[:, tt:tt + 1], axis=0))
            for dp in range(3):  # d-pairs share a 2-bank slot
                tp = psa()
                for k in range(2):
                    d = 2 * dp + k
                    sub = tp[:, k * 512:k * 512 + 128]
                    nc.tensor.transpose(out=sub, in_=xg[:, d * 128:(d + 1) * 128],
                                        identity=ident[:, :])
                    nc.vector.tensor_tensor(
                        out=xT[d][:, tt * 128:(tt + 1) * 128], in0=sub,
                        in1=posv[d][:, tt * 128:(tt + 1) * 128], op=ALU.add)

        # ================= layers =================
        for l in range(L):
            last = l == L - 1
            def _stop(tag):
                return last and stop_at == tag
            # ---- LN1 -> hT ----
            layernorm_T(hT)
            if _stop("ln1"):
                return nc

            bq = biasp.tile([128, 12], F32, name="bq", tag="bq")
            nc.sync.dma_start(out=bq[:, :], in_=bqkv[l][:, :])
            bv = biasp.tile([1, D], F32, name="bv", tag="bv")
            nc.sync.dma_start(out=bv[:, :], in_=bqv[l][:, :])
            for d in range(KD):
                nc.sync.dma_start(out=wbig[d][:, :D3], in_=wqkv[l][d * 128:(d + 1) * 128, :])
            # V staging buffers (one per head-half): set the per-head softmax-
            # ones columns now, on the gpsimd queue BEFORE the AG triggers
            # (whose engine-side waits would otherwise delay the V phase).
            vbig = [rot.tile([128, NT * VH], BF16, name=f"vbig{i}", tag=f"vbig{i}",
                             bufs=1) for i in range(2)]
            vbig4 = [v[:, :].rearrange("p (t h e) -> p t h e", t=NT, h=6)
                     for v in vbig]
            for i in range(2):
                nc.gpsimd.memset(vbig4[i][:, :, :, 64:65], 1.0)

            # ---- per head-half: K^T, V, staging DMA, AllGather ----
            # AG(half0) overlaps K/V(half1) + Q; AG(half1) overlaps attention
            # on half0.
            bvb = rot.tile([128, D], F32, name="bvb", tag="bvb", bufs=1)
            for vh in range(2):
                bcv = psy()
                nc.tensor.matmul(out=bcv[:, 0:384], lhsT=ones_row[:, :],
                                 rhs=bv[:, vh * 384:(vh + 1) * 384], start=True, stop=True)
                nc.vector.tensor_copy(out=bvb[:, vh * 384:(vh + 1) * 384],
                                      in_=bcv[:, 0:384])
            kv_in = [dram.tile([1, TOT2], BF16, name=f"kv_in{i}", tag=f"kv_in{i}")
                     for i in range(2)]
            kv_out = [dram.tile([1, NCORES * TOT2], BF16, name=f"kv_out{i}",
                                tag=f"kv_out{i}", addr_space="Shared")
                      for i in range(2)]
            kvof = [t[:, :].rearrange("o n -> (o n)") for t in kv_out]
            for hk in range(2):
                kvf = kv_in[hk][:, :].rearrange("o n -> (o n)")
                kbig = rot.tile([128, 3 * PT], BF16, name="kbig", tag=f"kbig{hk}",
                                bufs=1)
                for ot in range(6 + 3 * hk, 9 + 3 * hk):
                    ps = psa()
                    pv = ps[:, 0:PT]
                    for d in range(KD):
                        nc.tensor.matmul(out=pv, lhsT=wbig[d][:, ot * 128:(ot + 1) * 128],
                                         rhs=hT[d][:, :], start=(d == 0), stop=(d == KD - 1))
                    nc.scalar.activation(
                        out=kbig[:, (ot - 6 - 3 * hk) * PT:(ot - 5 - 3 * hk) * PT],
                        in_=pv, func=AF.Identity, bias=bq[:, ot:ot + 1])
                nc.sync.dma_start(
                    out=kvf[0:KT2].rearrange("(r p q) -> p r q", p=128, q=PT),
                    in_=kbig[:, :].rearrange("p (r q) -> p r q", q=PT))
                for tt in range(NT):
                    ps = psy()
                    for d in range(KD):
                        nc.tensor.matmul(
                            out=ps[:, 0:384],
                            lhsT=hT[d][:, tt * 128:(tt + 1) * 128],
                            rhs=wbig[d][:, D3 - D + hk * 384: D3 - D + (hk + 1) * 384],
                            start=(d == 0), stop=(d == KD - 1))
                    nc.vector.tensor_tensor(
                        out=vbig4[hk][:, tt, :, 0:64],
                        in0=ps[:, 0:384].rearrange("p (h e) -> p h e", e=64),
                        in1=bvb[:, hk * 384:(hk + 1) * 384].rearrange("p (h e) -> p h e", e=64),
                        op=ALU.add)
                nc.sync.dma_start(
                    out=kvf[KT2:TOT2].rearrange("(t p e) -> p t e", p=128, e=VH),
                    in_=vbig[hk][:, :].rearrange("p (t e) -> p t e", e=VH))
                nc.gpsimd.collective_compute(
                    "AllGather", ALU.bypass,
                    replica_groups=[list(range(NCORES))],
                    ins=[kv_in[hk][:, :].opt()], outs=[kv_out[hk][:, :].opt()])

            # ---- Q^T (overlaps the AllGathers) ----
            for ot in range(6):
                ps = psa()
                pv = ps[:, 0:PT]
                for d in range(KD):
                    nc.tensor.matmul(out=pv, lhsT=wbig[d][:, ot * 128:(ot + 1) * 128],
                                     rhs=hT[d][:, :], start=(d == 0), stop=(d == KD - 1))
                nc.scalar.activation(out=qT[ot][:, :], in_=pv,
                                     func=AF.Identity, bias=bq[:, ot:ot + 1])
            if _stop("qkv"):
                return nc

            # prefetch next-phase weights under attention
            for k in range(KD):
                nc.sync.dma_start(out=w768[k][:, :], in_=wout[l][k * 128:(k + 1) * 128, :])
            for d in range(KD):
                nc.sync.dma_start(out=wbig[d][:, :], in_=w1[l][d * 128:(d + 1) * 128, :])
            bft = biasp.tile([128, 24], F32, name="bft", tag="bft")
            nc.sync.dma_start(out=bft[:, :], in_=b1[l][:, :])

            # ---- attention, half the heads at a time ----
            for half in range(2):
                for c in range(NCORES):
                    nc.sync.dma_start(
                        out=ktg[c][:, :].rearrange("p (r q) -> p r q", q=PT),
                        in_=kvof[half][c * TOT2:c * TOT2 + KT2]
                        .rearrange("(r p q) -> p r q", p=128, q=PT))
                    nc.sync.dma_start(
                        out=vg[c][:, :].rearrange("p (t e) -> p t e", e=VH),
                        in_=kvof[half][c * TOT2 + KT2:(c + 1) * TOT2]
                        .rearrange("(t p e) -> p t e", p=128, e=VH))
                for pr in range(3):
                    h0 = half * 6 + 2 * pr
                    qtile = qT[h0 // 2]
                    y_ps = [psy() for _ in range(2)]  # per head in pair
                    for c in range(NCORES):
                        S = [psa() for _ in range(2)]
                        for b in range(2):
                            for j in range(2):
                                for hh in range(2):
                                    kp = hh * 64
                                    nc.tensor.matmul(
                                        out=S[hh][:, b * 512 + j * 256:
                                                  b * 512 + j * 256 + 256 - j * 128],
                                        lhsT=ktg[c][kp:kp + 64,
                                                    pr * PT + (2 * b + j) * 128:
                                                    pr * PT + (2 * b + j + 1) * 128],
                                        rhs=qtile[kp:kp + 64,
                                                  b * TB + j * 128:(b + 1) * TB],
                                        start=(j == 0), stop=(j == 1))
                        for hh in range(2):
                            es = esp.tile([128, 2 * 384], BF16, name="es", tag="es")
                            nc.scalar.activation(
                                out=es[:, :].rearrange("p (b q) -> p b q", b=2),
                                in_=S[hh][:, :].rearrange("p (b q) -> p b q", b=2)[:, :, 0:384],
                                func=AF.Exp, bias=zero_col[:, :], scale=0.125)
                            es4 = es[:, :].rearrange("p (b j q) -> p b j q", b=2, q=128)
                            nc.gpsimd.tensor_tensor(
                                out=es4[:, :, 0::2, :], in0=es4[:, :, 0::2, :],
                                in1=mask_sb[:, c * 128:(c + 1) * 128]
                                .rearrange("p (b j q) -> p b j q", b=1, j=1)
                                .broadcast_to([128, 2, 2, 128]),
                                op=ALU.mult)
                            hs = 2 * pr + hh
                            es2 = es[:, :].rearrange("p (b q) -> p b q", b=2)
                            for b in range(2):
                                nc.tensor.matmul(
                                    out=y_ps[hh][0:VW, b * TB:(b + 1) * TB],
                                    lhsT=vg[c][:, (2 * b) * VH + hs * VW:
                                               (2 * b) * VH + (hs + 1) * VW],
                                    rhs=es2[:, b, 0:256],
                                    start=(c == 0 and b == 0), stop=False)
                                nc.tensor.matmul(
                                    out=y_ps[hh][0:VW, b * TB + 128:(b + 1) * TB],
                                    lhsT=vg[c][:, (2 * b + 1) * VH + hs * VW:
                                               (2 * b + 1) * VH + (hs + 1) * VW],
                                    rhs=es2[:, b, 256:384],
                                    start=False, stop=(c == NCORES - 1 and b == 1))
                    # normalize pair
                    for hh in range(2):
                        h = h0 + hh
                        zrec = rowp.tile([1, PT], F32, name="zrec", tag="row")
                        nc.vector.reciprocal(out=zrec[:, :], in_=y_ps[hh][64:65, :])
                        bcb = psa()
                        nc.tensor.matmul(out=bcb[0:64, 0:PT], lhsT=ones_row[:, 0:64],
                                         rhs=zrec[:, :], start=True, stop=True)
                        bcs = rot.tile([64, PT], BF16, name="bcs", tag="bcs")
                        nc.vector.tensor_copy(out=bcs[:, :], in_=bcb[0:64, 0:PT])
                        nc.vector.tensor_tensor(
                            out=yT[h // 2][(h % 2) * 64:(h % 2) * 64 + 64, :],
                            in0=y_ps[hh][0:64, :], in1=bcs[:, :], op=ALU.mult)

            if _stop("attn"):
                return nc
            # ---- out_proj + residual: xT += Wout^T y^T ----
            oslot = [psa() for _ in range(3)]
            oacc = [oslot[o // 2][:, (o % 2) * 512:(o % 2) * 512 + PT] for o in range(KD)]
            for k in range(KD):
                for o in range(KD):
                    nc.tensor.matmul(out=oacc[o], lhsT=w768[k][:, o * 128:(o + 1) * 128],
                                     rhs=yT[k][:, :], start=(k == 0), stop=(k == KD - 1))
            for o in range(KD):
                nc.vector.tensor_add(out=xT[o][:, :], in0=xT[o][:, :], in1=oacc[o])
            if _stop("proj"):
                return nc

            # ---- LN2 -> hT ----
            layernorm_T(hT)

            # ---- FF: g^T tile-by-tile, immediately consumed into ff2 accumulators ----
            fslot = [psa() for _ in range(3)]
            facc = [fslot[o // 2][:, (o % 2) * 512:(o % 2) * 512 + PT] for o in range(KD)]
            for ot in range(24):
                ps = psy()
                for d in range(KD):
                    nc.tensor.matmul(out=ps[:, :], lhsT=wbig[d][:, ot * 128:(ot + 1) * 128],
                                     rhs=hT[d][:, :], start=(d == 0), stop=(d == KD - 1))
                g = gp.tile([128, PT], BF16, name="g", tag="g")
                nc.scalar.activation(out=g[:, :], in_=ps[:, :], func=AF.Gelu,
                                     bias=bft[:, ot:ot + 1], scale=1.0)
                wslab = w2p.tile([128, D], BF16, name="w2s", tag="w2s")
                nc.sync.dma_start(out=wslab[:, :], in_=w2[l][ot * 128:(ot + 1) * 128, :])
                for o in range(KD):
                    nc.tensor.matmul(out=facc[o], lhsT=wslab[:, o * 128:(o + 1) * 128],
                                     rhs=g[:, :], start=(ot == 0), stop=(ot == 23))
            for o in range(KD):
                nc.vector.tensor_add(out=xT[o][:, :], in0=xT[o][:, :], in1=facc[o])

        # ================= final LN + lm_head =================
        layernorm_T(hT)
        for vc in range(NVC):
            nv = min(512, V - vc * 512)
            esl = embp.tile([128, KD * 512], BF16, name="esl", tag="esl")
            nc.sync.dma_start(
                out=esl[:, :].rearrange("p (d v) -> p d v", v=512)[:, :, 0:nv],
                in_=embT[:, vc * 512:vc * 512 + nv]
                .rearrange("(d p) v -> p d v", p=128))
            esl3 = esl[:, :].rearrange("p (d v) -> p d v", v=512)
            lsb = logp.tile([128, NT * 512], BF16, name="lsb", tag="lsb")
            lsb3 = lsb[:, :].rearrange("p (t v) -> p t v", v=512)
            for tp2 in range(2):
                ps = psa()
                for k in range(2):
                    tt = 2 * tp2 + k
                    pv = ps[:, k * 512:k * 512 + nv]
                    for d in range(KD):
                        nc.tensor.matmul(out=pv,
                                         lhsT=hT[d][:, tt * 128:(tt + 1) * 128],
                                         rhs=esl3[:, d, 0:nv], start=(d == 0),
                                         stop=(d == KD - 1))
                    if tt % 2 == 0:
                        nc.vector.tensor_copy(out=lsb3[:, tt, 0:nv], in_=pv)
                    else:
                        nc.scalar.activation(out=lsb3[:, tt, 0:nv], in_=pv,
                                             func=AF.Identity, bias=zero_col[:, :])
            nc.sync.dma_start(
                out=logits[:, vc * 512:vc * 512 + nv]
                .rearrange("(t p) v -> p t v", p=128),
                in_=lsb3[:, :, 0:nv])
    nc.finalize()
    return nc


# ------------------------------------------------------------------
# host side
# ------------------------------------------------------------------

def _prep_inputs(nb, L, V, idx, tok_emb, pos_emb, ln1_w, ln1_b, qkv_w, out_w,
                 ln2_w, ln2_b, ff1_w, ff2_w, lnf_w, lnf_b):
    NT = 2 * nb
    PT = NT * 128
    idx = np.asarray(idx).astype(np.int32)
    f = np.asarray

    shared = {
        "toke": f(tok_emb, dtype=np.float32),
        "embT": np.ascontiguousarray((f(tok_emb, dtype=np.float32) * f(lnf_w, dtype=np.float32)[None, :]).T).astype(bf16),
    }
    for l in range(L):
        wq = f(qkv_w[l], dtype=np.float32) * f(ln1_w[l], dtype=np.float32)[:, None]
        bq_full = f(ln1_b[l], dtype=np.float32) @ f(qkv_w[l], dtype=np.float32)  # [3D]
        shared[f"wqkv{l}"] = wq.astype(bf16)
        shared[f"bqkv{l}"] = np.ascontiguousarray(bq_full[:12 * 128].reshape(12, 128).T).astype(np.float32)
        shared[f"bqv{l}"] = bq_full[2 * D:].reshape(1, D).astype(np.float32)
        shared[f"wout{l}"] = f(out_w[l], dtype=np.float32).astype(bf16)
        w1e = f(ff1_w[l], dtype=np.float32) * f(ln2_w[l], dtype=np.float32)[:, None]
        b1_full = f(ln2_b[l], dtype=np.float32) @ f(ff1_w[l], dtype=np.float32)  # [4D]
        shared[f"w1_{l}"] = w1e.astype(bf16)
        shared[f"b1_{l}"] = np.ascontiguousarray(b1_full.reshape(24, 128).T).astype(np.float32)
        shared[f"w2_{l}"] = f(ff2_w[l], dtype=np.float32).astype(bf16)

    pos_f = f(pos_emb, dtype=np.float32)
    in_maps = []
    for c in range(NCORES):
        m = dict(shared)
        L_loc = np.arange(PT)
        b_loc = L_loc // (nb * 128)
        t_loc = 8 * (L_loc % (nb * 128)) + c
        idx_core = idx[b_loc, t_loc]  # [PT]
        m["idxs"] = np.ascontiguousarray(idx_core.reshape(NT, 128).T).astype(np.int32)
        m["posT"] = np.ascontiguousarray(pos_f[t_loc].T).astype(bf16)
        mk = np.zeros((128, NCORES * 128), dtype=np.float32)
        for cp in range(NCORES):
            mk[:, cp * 128:(cp + 1) * 128] = np.triu(np.ones((128, 128), np.float32),
                                                     0 if cp <= c else 1)
        m["masks"] = mk.astype(bf16)
        in_maps.append(m)
    return in_maps


_NC_CACHE = {}


def _get_nc(nb, L, V):
    key = (nb, L, V)
    if key not in _NC_CACHE:
        _NC_CACHE[key] = build_nc(nb, L, V)
    return _NC_CACHE[key]


def run_on_hw(nb, L, V, inputs, trace=False):
    from concourse import bass_utils
    nc = _get_nc(nb, L, V)
    in_maps = _prep_inputs(nb, L, V, **inputs)
    res = bass_utils.run_bass_kernel_spmd(nc, in_maps, core_ids=list(range(NCORES)),
                                          trace=trace)
    return res


def assemble(nb, L, V, results, lnf_b, tok_emb):
    T = 8 * nb * 128
    out = np.empty((2, T, V), dtype=np.float32)
    for c in range(NCORES):
        lg = results[c]["logits"].astype(np.float32).reshape(2, nb * 128, V)
        out[:, c::8, :] = lg
    lnf_b = np.asarray(lnf_b, dtype=np.float32)
    if np.any(lnf_b):
        out += (lnf_b @ np.asarray(tok_emb, dtype=np.float32).T)[None, None, :]
    return out


def kernel(**inputs):
    nb, L, V = 2, 6, 32000
    res = run_on_hw(nb, L, V, inputs)
    return assemble(nb, L, V, res.results, inputs["lnf_b"], inputs["tok_emb"])
